# revision 1
# baseline (speedup 1.0000x reference)
"""Lp-distance (p=8) BasicBlock kernel for 8 Trainium2 NeuronCores.

Math (per conv, per output channel o), with mid=(pl+pu)/2, h=(pu-pl)/2 and
t = |w - mid| per patch element:
    value = (sum_ckk (patch_x - w[o])^8)^(1/8)          (binomial, on PE)
    dl    = (sum_ckk relu(t - h)^8)^(1/8)               (fp16 chains, DVE+ACT)
    du    = (sum_ckk (t + h)^8)^(1/8)
The first bound_relu is an exact no-op (all conv outputs are >= 0), so conv2
consumes conv1 outputs directly.

Sharding: (batch=4) x (H-halves=2) -> 8 cores, zero collectives.  Each core
gets host-padded mid/h/x slabs with a 2-row halo, computes conv1 on 18 rows
(one fictional edge row zeroed via the Lp-root's exp bias = -1e30 mask),
bounces conv1 results through DRAM canvases, computes conv2 on 16 rows, adds
the residual (actual lower/upper inputs) and final relu.

Bounds layout: hw-positions on partitions, (o, ckk) on the free dim; heavy
chains are fat [pw, 32*288] fp16 passes (DVE tensor_tensor at 2x, ACT Square)
with per-o sums via scalar_tensor_tensor's fused fp32 accumulator; the weight
operand broadcasts via a step-0 AP dim.  Value path: fp32 binomial expansion
sum_j C(8,j) px^j (-w)^(8-j) as 24 accumulating matmuls on the otherwise-idle
TensorE in (c, hw) layout, j=0 term folded into the Ln bias.  Lp root =
exp(ln(z)/8) on ACT; Ln/Exp/Square/Abs/Relu all live in one ACT table set.

Uniform-h fast path (_build_u2): when upper-lower is a uniform 2*h0 (the
harness always generates lower=x-eps, upper=x+eps), conv1's h is baked as an
immediate: no hp canvas / h-patch DMAs, q-chain starts with one fused
tensor_scalar (b-h0 then relu), s-chain with one ACT Square(b + h0*bias).
kernel() detects uniformity at runtime and falls back to the general build
otherwise.

Toolchain notes: this walrus build allows at most one sync-wait per
instruction (see _split_multiwait) and supports no custom-DVE ops; AluOpType
pow/abs_max fail codegen; tensor_scalar two-op fusions and ACT bias tiles
are safe.
"""
import json

import ml_dtypes
import numpy as np

import concourse.bass as bass
import concourse.bass2jax as bass2jax
import concourse.bass_utils as bass_utils
import concourse.mybir as mybir
import concourse.tile as tile
from concourse.bass import AP
from concourse.bass_utils import run_bass_kernel_spmd

# ---------------------------------------------------------------------------
# Walrus workaround: this toolchain's codegen accepts at most ONE sync-wait
# per instruction; Tile emits several on drains/joins.  Split the extras onto
# preceding same-engine NoOps (semantically identical: waits run in order).
_orig_cbk = bass_utils.compile_bir_kernel


def _split_multiwait(bir_bytes):
    bir = json.loads(bir_bytes)
    ctr = 0
    for f in bir.get("functions", []):
        for blk in f.get("blocks", []):
            out = []
            for ins in blk["instructions"]:
                si = ins.get("sync_info")
                ow = (si or {}).get("on_wait") or []
                if len(ow) > 1:
                    si["on_wait"] = ow[-1:]
                    for w in ow[:-1]:
                        ctr += 1
                        out.append({
                            "debug": ins.get("debug", 0),
                            "engine": ins["engine"], "ins": [],
                            "name": f"I-WSPLIT{ctr}", "opcode": "NoOp",
                            "outs": [],
                            "sync_info": {"on_wait": [w], "on_update": []}})
                out.append(ins)
            blk["instructions"][:] = out
    return json.dumps(bir).encode()


def _patched_cbk(bir_json, tmpdir, neff_name="file.neff"):
    return _orig_cbk(_split_multiwait(bir_json), tmpdir, neff_name)


if bass_utils.compile_bir_kernel is not _patched_cbk:
    bass_utils.compile_bir_kernel = _patched_cbk
    bass2jax.compile_bir_kernel = _patched_cbk

# ---------------------------------------------------------------------------
F = mybir.ActivationFunctionType
A = mybir.AluOpType
DT = mybir.dt

CT = DT.float16             # compute dtype: same DVE speed tier as bf16, 8x finer
NPCT = np.float16
OG = 32                     # output channels per fat pass (fat width = OG*288)
EPS = 0.1
NEGINF = -1e30              # exp(x + NEGINF) == 0 in fp32

B, C, H, W = 4, 32, 32, 32
CKK = 288                   # 3*3*32, ordered (dy, dx, c)
ROWS1 = 18                  # conv1 output rows per core (incl 1 fictional)
ROWS2 = 16                  # conv2 output rows per core
HW1 = ROWS1 * 32            # 576
HW2 = ROWS2 * 32            # 512
CV_ROWSTRIDE = 34 * 32      # canvas row stride in elements


def _hwtiles(hw):
    """[(p0, pw), ...] partition tiles covering hw positions."""
    out = []
    p = 0
    while p < hw:
        pw = min(128, hw - p)
        out.append((p, pw))
        p += pw
    return out


def _patch_src_dy(t, y0, nrows, dy):
    """Overlapping 3x3 patch gather, one dy slice: out position (y,x) reads
    row y0+y+dy, cols x..x+2, all c (free order (dx, c), contiguous 96)."""
    return AP(tensor=t, offset=(y0 + dy) * CV_ROWSTRIDE,
              ap=[[CV_ROWSTRIDE, nrows], [32, 32], [1, 96]])


def _dma_patch(nc, dst, src_t, y0, nrows):
    for dy in range(3):
        nc.sync.dma_start(dst[:, dy * 96:(dy + 1) * 96],
                          _patch_src_dy(src_t, y0, nrows, dy))


def _canvas_interior(t, y0, nrows):
    """Write [nrows*32, 32] (hw, c) into canvas rows y0.., cols 1..32."""
    return AP(tensor=t, offset=(y0 * 34 + 1) * 32,
              ap=[[CV_ROWSTRIDE, nrows], [32, 32], [1, 32]])


def _rep(ap_, n):
    """Repeat a [P, W] AP n times along a step-0 middle dim -> [P, n, W]."""
    return AP(tensor=ap_.tensor, offset=ap_.offset,
              ap=[list(ap_.ap[0]), [0, n], list(ap_.ap[-1])])


def _build(repeat=1):
    nc = bass.Bass("TRN2", target_bir_lowering=False, debug=False,
                   num_devices=8)
    mp = nc.dram_tensor("mp", [20, 34, 32], CT, kind="ExternalInput")
    hp = nc.dram_tensor("hp", [20, 34, 32], CT, kind="ExternalInput")
    xpc = nc.dram_tensor("xpc", [32, 20, 34], DT.float32, kind="ExternalInput")
    lch = nc.dram_tensor("lch", [HW2, 32], DT.float32, kind="ExternalInput")
    uch = nc.dram_tensor("uch", [HW2, 32], DT.float32, kind="ExternalInput")
    xcc = nc.dram_tensor("xcc", [32, HW2], DT.float32, kind="ExternalInput")
    lmask = nc.dram_tensor("lmask", [HW1, 1], DT.float32, kind="ExternalInput")
    vmask = nc.dram_tensor("vmask", [32, HW1], DT.float32, kind="ExternalInput")
    wb1 = nc.dram_tensor("wb1", [128, 32 * CKK], CT, kind="ExternalInput")
    wb2 = nc.dram_tensor("wb2", [128, 32 * CKK], CT, kind="ExternalInput")
    wj1 = nc.dram_tensor("wj1", [8, CKK, 32], DT.float32, kind="ExternalInput")
    wj2 = nc.dram_tensor("wj2", [8, CKK, 32], DT.float32, kind="ExternalInput")
    b01 = nc.dram_tensor("b01", [32, 1], DT.float32, kind="ExternalInput")
    b02 = nc.dram_tensor("b02", [32, 1], DT.float32, kind="ExternalInput")
    cm = nc.dram_tensor("cm", [ROWS1, 34, 32], CT)
    ch = nc.dram_tensor("ch", [ROWS1, 34, 32], CT)
    cvv = nc.dram_tensor("cvv", [32, ROWS1, 34], DT.float32)
    out_b = nc.dram_tensor("out_b", [2, HW2, 32], DT.float32,
                           kind="ExternalOutput")
    out_v = nc.dram_tensor("out_v", [32, HW2], DT.float32,
                           kind="ExternalOutput")

    FATW = 32 * CKK

    with tile.TileContext(nc) as tc:
        with (
            tc.tile_pool(name="const", bufs=1) as constp,
            tc.tile_pool(name="wpool", bufs=1) as wpool,
            tc.tile_pool(name="patch", bufs=2) as patchp,
            tc.tile_pool(name="fat", bufs=2) as fatp,
            tc.tile_pool(name="small", bufs=3) as smallp,
            tc.tile_pool(name="vpow", bufs=1) as vpowp,
            tc.tile_pool(name="psum", bufs=2, space="PSUM") as psump,
        ):
            twb = wpool.tile([128, FATW], CT, name="twb")
            nc.sync.dma_start(twb[:], wb1.ap())
            # value lhsT chunks loaded lazily at first use (keeps startup
            # DMA on the bounds-critical path)
            twjs = {}
            tb0 = {}

            def load_value_weights(cv):
                wjt = {1: wj1, 2: wj2}[cv]
                bt = {1: b01, 2: b02}[cv]
                for j in range(8):
                    for ck in range(3):
                        t = wpool.tile([96, 32], DT.float32,
                                       name=f"wj{cv}_{j}_{ck}")
                        nc.sync.dma_start(
                            t[:], wjt.ap()[j, ck * 96:(ck + 1) * 96, :])
                        twjs[(cv, j, ck)] = t
                t = wpool.tile([32, 1], DT.float32, name=f"b0_{cv}")
                nc.sync.dma_start(t[:], bt.ap())
                tb0[cv] = t
            zcol = constp.tile([128, 1], DT.float32, name="zcol")
            nc.gpsimd.memset(zcol[:], 0.0)
            nepscol = constp.tile([128, 1], DT.float32, name="nepscol")
            nc.gpsimd.memset(nepscol[:], -EPS)
            pepscol = constp.tile([128, 1], DT.float32, name="pepscol")
            nc.gpsimd.memset(pepscol[:], EPS)
            zfill = constp.tile([128, 153], CT, name="zfill")
            nc.gpsimd.memset(zfill[:], 0.0)
            zfill32 = constp.tile([128, 153], DT.float32, name="zfill32")
            nc.gpsimd.memset(zfill32[:], 0.0)
            for _repidx in range(repeat):
                for t in (cm, ch):
                    nc.sync.dma_start(
                        AP(tensor=t, offset=0, ap=[[1, ROWS1 * 34 * 32]]),
                        zfill[:])
                # exact-count zero fill for cvv: 32*18*34 = 19584 = 128*153
                nc.sync.dma_start(
                    AP(tensor=cvv, offset=0, ap=[[1, 32 * ROWS1 * 34]]),
                    zfill32[:])

                sttscr = constp.tile([128, CKK], CT, name="sttscr")

                def bounds_tile(conv, p0, pw, wtile, pm, ph, lm):
                    """Bounds chains for one hw-tile; returns (dl_rt, du_rt)."""
                    a = fatp.tile([128, FATW], CT, name="fatA", tag="fatA")
                    b = fatp.tile([128, FATW], CT, name="fatB", tag="fatB")
                    c = fatp.tile([128, FATW], CT, name="fatC", tag="fatC",
                                  bufs=1)
                    d = fatp.tile([128, FATW], CT, name="fatD", tag="fatD",
                                  bufs=1)  # bufs kept at 1: SBUF-bound
                    e = fatp.tile([128, FATW], CT, name="fatE", tag="fatE",
                                  bufs=1)
                    zl = smallp.tile([128, 32], DT.float32, name="zl", tag="zl")
                    zu = smallp.tile([128, 32], DT.float32, name="zu", tag="zu")
                    a, b, c, d, e = a[:pw], b[:pw], c[:pw], d[:pw], e[:pw]
                    zl, zu = zl[:pw], zu[:pw]
                    scr = sttscr[:pw]

                    def sl(t, o):
                        return t[:, o * CKK:(o + 1) * CKK]

                    def as3(t):
                        return AP(tensor=t.tensor, offset=t.offset,
                                  ap=[list(t.ap[0]), [CKK, 32], [1, CKK]])

                    # t = pm(rep) - w ; at = |t|
                    nc.vector.tensor_tensor(as3(a), _rep(pm, 32),
                                            as3(wtile[:pw]), A.subtract)
                    nc.scalar.activation(b[:], a[:], F.Abs)
                    # chains (tensor h for both convs)
                    nc.vector.tensor_tensor(as3(c), as3(b), _rep(ph, 32),
                                            A.subtract)             # q
                    nc.vector.tensor_tensor(as3(d), as3(b), _rep(ph, 32),
                                            A.add)                  # s
                    nc.vector.tensor_scalar(c[:], c[:], 0.0, None, A.max)
                    nc.scalar.activation(e[:], c[:], F.Square)      # r2
                    nc.scalar.activation(c[:], e[:], F.Square)      # r4
                    nc.scalar.activation(e[:], d[:], F.Square)      # s2
                    if conv == 1:
                        nc.vector.tensor_tensor(d[:], e[:], e[:], A.mult)
                    else:
                        nc.scalar.activation(d[:], e[:], F.Square)  # s4
                    rsum, ssum = c, d
                    for o in range(32):
                        nc.vector.scalar_tensor_tensor(
                            scr[:], sl(rsum, o), 0.0, sl(rsum, o), A.add, A.mult,
                            accum_out=zl[:, o:o + 1])
                    for o in range(32):
                        nc.vector.scalar_tensor_tensor(
                            scr[:], sl(ssum, o), 0.0, sl(ssum, o), A.add, A.mult,
                            accum_out=zu[:, o:o + 1])
                    rdt = CT if conv == 1 else DT.float32
                    bias = lm if conv == 1 else zcol[:pw]
                    roots = []
                    for z in (zl, zu):
                        lnz = smallp.tile([128, 32], DT.float32, name="lnz",
                                          tag="lnz")[:pw]
                        rt = smallp.tile([128, 32], rdt, name=f"rt{conv}",
                                         tag=f"rt{conv}")[:pw]
                        nc.scalar.activation(lnz[:], z[:], F.Ln)
                        nc.scalar.activation(rt[:], lnz[:], F.Exp, bias=bias[:],
                                             scale=0.125)
                        roots.append(rt)
                    return roots

                def value_conv(conv, src_dram, hw, wtile_key, mask):
                    """Binomial value path in (c, hw) layout via PE.
                    Returns y [32, hw] f32 SBUF tile (masked for conv1)."""
                    nrows = hw // 32
                    # patch chunks [96, hw] x3 (dy), rows (dx, c) ordered... NOTE:
                    # chunk rows must match wj ordering (dy, dx, c) c-minor.
                    px = []
                    for dy in range(3):
                        t = vpowp.tile([96, hw], DT.float32, name=f"px{dy}",
                                       tag=f"px{dy}")
                        for dx in range(3):
                            src = AP(tensor=src_dram,
                                     offset=dy * 34 + dx,
                                     ap=[[20 * 34 if conv == 1 else ROWS1 * 34, 32],
                                         [34, nrows], [1, 32]])
                            nc.sync.dma_start(t[dx * 32:(dx + 1) * 32, :], src)
                        px.append(t)
                    nps = (hw + 511) // 512
                    psums = [psump.tile([32, min(512, hw - i * 512)], DT.float32,
                                        name=f"vps{i}", tag=f"vps{i}")
                             for i in range(nps)]

                    def mm(j, ck, t, start):
                        for i, ps in enumerate(psums):
                            nc.tensor.matmul(
                                ps[:], twjs[(conv, j, ck)][:],
                                t[:, i * 512:i * 512 + ps.shape[1]],
                                start=start, stop=(j == 7))
                    for ck in range(3):
                        p1 = px[ck]
                        p2 = vpowp.tile([96, hw], DT.float32, name="p2", tag="p2")
                        p4 = vpowp.tile([96, hw], DT.float32, name="p4", tag="p4")
                        tmp = vpowp.tile([96, hw], DT.float32, name="tmp",
                                         tag="tmp")
                        mm(0, ck, p1, start=(ck == 0))          # j index 0 == x^1
                        nc.vector.tensor_tensor(p2[:], p1[:], p1[:], A.mult)
                        mm(1, ck, p2, start=False)
                        nc.vector.tensor_tensor(tmp[:], p2[:], p1[:], A.mult)
                        mm(2, ck, tmp, start=False)             # x^3
                        nc.vector.tensor_tensor(p4[:], p2[:], p2[:], A.mult)
                        mm(3, ck, p4, start=False)
                        nc.vector.tensor_tensor(tmp[:], p4[:], p1[:], A.mult)
                        mm(4, ck, tmp, start=False)             # x^5
                        nc.vector.tensor_tensor(tmp[:], p4[:], p2[:], A.mult)
                        mm(5, ck, tmp, start=False)             # x^6
                        nc.vector.tensor_tensor(tmp[:], tmp[:], p1[:], A.mult)
                        mm(6, ck, tmp, start=False)             # x^7
                        nc.vector.tensor_tensor(tmp[:], p4[:], p4[:], A.mult)
                        mm(7, ck, tmp, start=False)             # x^8
                    y = smallp.tile([32, HW1], DT.float32, name=f"yv{conv}",
                                    tag=f"yv{conv}")[:, :hw]
                    for i, ps in enumerate(psums):
                        w = ps.shape[1]
                        seg = y[:, i * 512:i * 512 + w]
                        nc.scalar.activation(seg, ps[:], F.Relu)
                        nc.scalar.activation(seg, seg, F.Ln, bias=tb0[conv][:])
                        nc.scalar.activation(seg, seg, F.Exp, scale=0.125)
                    if mask is not None:
                        nc.vector.tensor_tensor(y[:], y[:], mask, A.mult)
                    return y

                # ================= conv1 =================
                y1v = None
                for ti, (p0, pw) in enumerate(_hwtiles(HW1)):
                    y0 = p0 // 32
                    nrows = pw // 32
                    pm = patchp.tile([128, CKK], CT, name="pm1", tag="pm1")[:pw]
                    _dma_patch(nc, pm, mp, y0, nrows)
                    ph1 = patchp.tile([128, CKK], CT, name="ph1",
                                      tag="ph1")[:pw]
                    _dma_patch(nc, ph1, hp, y0, nrows)
                    lm = smallp.tile([128, 1], DT.float32, name="lm",
                                     tag="lm")[:pw]
                    nc.sync.dma_start(lm[:], lmask.ap()[p0:p0 + pw, :])
                    dl1, du1 = bounds_tile(1, p0, pw, twb, pm, ph1, lm)
                    m2 = smallp.tile([128, 32], CT, name="m2", tag="m2")[:pw]
                    h2 = smallp.tile([128, 32], CT, name="h2", tag="h2")[:pw]
                    nc.vector.tensor_tensor(m2[:], dl1[:], du1[:], A.add)
                    nc.vector.tensor_scalar(m2[:], m2[:], 0.5, None, A.mult)
                    nc.vector.tensor_tensor(h2[:], du1[:], dl1[:], A.subtract)
                    nc.vector.tensor_scalar(h2[:], h2[:], 0.5, None, A.mult)
                    nc.sync.dma_start(_canvas_interior(cm, y0, nrows), m2[:])
                    nc.sync.dma_start(_canvas_interior(ch, y0, nrows), h2[:])
                    if ti == 0:
                        # value path (c, hw) via PE, interleaves with bounds
                        load_value_weights(1)
                        y1v = value_conv(1, xpc, HW1, 1, None)
                        vm = smallp.tile([32, HW1], DT.float32, name="vm",
                                         tag="vm")
                        nc.sync.dma_start(vm[:], vmask.ap())
                        nc.vector.tensor_tensor(y1v[:], y1v[:], vm[:], A.mult)
                        nc.sync.dma_start(
                            AP(tensor=cvv, offset=1,
                               ap=[[ROWS1 * 34, 32], [34, ROWS1], [1, 32]]),
                            y1v[:])
                # reload shared weight tile for conv2
                nc.sync.dma_start(twb[:], wb2.ap())

                # ================= conv2 =================
                load_value_weights(2)
                y2v = value_conv(2, cvv, HW2, 2, None)
                xcct = smallp.tile([32, HW2], DT.float32, name="xcct", tag="xcct")
                nc.sync.dma_start(xcct[:], xcc.ap())
                nc.vector.tensor_tensor(y2v[:], y2v[:], xcct[:], A.add)
                nc.scalar.activation(y2v[:], y2v[:], F.Relu)
                nc.sync.dma_start(out_v.ap(), y2v[:])
                tiles2 = _hwtiles(HW2)
                heads = {}
                for ti in range(len(tiles2) + 1):
                    if ti < len(tiles2):
                        p0, pw = tiles2[ti]
                        pmid = patchp.tile([128, CKK], CT, name="pmid",
                                           tag="pmid")[:pw]
                        phh = patchp.tile([128, CKK], CT, name="phh",
                                          tag="phh")[:pw]
                        _dma_patch(nc, pmid, cm, p0 // 32, pw // 32)
                        _dma_patch(nc, phh, ch, p0 // 32, pw // 32)
                        heads[ti] = (p0, pw, bounds_head(pw, pmid), phh)
                    if ti - 1 not in heads:
                        continue
                    p0, pw, btl, phh = heads.pop(ti - 1)
                    dl2, du2 = bounds_tail(2, pw, btl, phh, None)
                    lct = smallp.tile([128, 32], DT.float32, name="lct",
                                      tag="lct")[:pw]
                    uct = smallp.tile([128, 32], DT.float32, name="uct",
                                      tag="uct")[:pw]
                    nc.sync.dma_start(lct[:], lch.ap()[p0:p0 + pw, :])
                    nc.sync.dma_start(uct[:], uch.ap()[p0:p0 + pw, :])
                    for k, (rt, resid) in enumerate(((dl2, lct), (du2, uct))):
                        ro = smallp.tile([128, 32], DT.float32, name="ro",
                                         tag="ro")[:pw]
                        nc.vector.tensor_tensor(ro[:], rt[:], resid[:], A.add)
                        nc.scalar.activation(ro[:], ro[:], F.Relu)
                        nc.sync.dma_start(out_b.ap()[k, p0:p0 + pw, :], ro[:])
    return nc



def _build_u2(h0, repeat=1):
    """Baseline structure with conv1 running on a scalar interval
    half-width h0: no hp canvas / ph patches; c-chain via one fused
    tensor_scalar (sub,max); s2 via ACT Square with +h0 bias."""
    nc = bass.Bass("TRN2", target_bir_lowering=False, debug=False,
                   num_devices=8)
    mp = nc.dram_tensor("mp", [20, 34, 32], CT, kind="ExternalInput")
    xpc = nc.dram_tensor("xpc", [32, 20, 34], DT.float32, kind="ExternalInput")
    lch = nc.dram_tensor("lch", [HW2, 32], DT.float32, kind="ExternalInput")
    uch = nc.dram_tensor("uch", [HW2, 32], DT.float32, kind="ExternalInput")
    xcc = nc.dram_tensor("xcc", [32, HW2], DT.float32, kind="ExternalInput")
    lmask = nc.dram_tensor("lmask", [HW1, 1], DT.float32, kind="ExternalInput")
    vmask = nc.dram_tensor("vmask", [32, HW1], DT.float32, kind="ExternalInput")
    wb1 = nc.dram_tensor("wb1", [128, 32 * CKK], CT, kind="ExternalInput")
    wb2 = nc.dram_tensor("wb2", [128, 32 * CKK], CT, kind="ExternalInput")
    wj1 = nc.dram_tensor("wj1", [8, CKK, 32], DT.float32, kind="ExternalInput")
    wj2 = nc.dram_tensor("wj2", [8, CKK, 32], DT.float32, kind="ExternalInput")
    b01 = nc.dram_tensor("b01", [32, 1], DT.float32, kind="ExternalInput")
    b02 = nc.dram_tensor("b02", [32, 1], DT.float32, kind="ExternalInput")
    cm = nc.dram_tensor("cm", [ROWS1, 34, 32], CT)
    ch = nc.dram_tensor("ch", [ROWS1, 34, 32], CT)
    cvv = nc.dram_tensor("cvv", [32, ROWS1, 34], DT.float32)
    out_b = nc.dram_tensor("out_b", [2, HW2, 32], DT.float32,
                           kind="ExternalOutput")
    out_v = nc.dram_tensor("out_v", [32, HW2], DT.float32,
                           kind="ExternalOutput")

    FATW = 32 * CKK

    with tile.TileContext(nc) as tc:
        with (
            tc.tile_pool(name="const", bufs=1) as constp,
            tc.tile_pool(name="wpool", bufs=1) as wpool,
            tc.tile_pool(name="patch", bufs=2) as patchp,
            tc.tile_pool(name="fat", bufs=2) as fatp,
            tc.tile_pool(name="small", bufs=3) as smallp,
            tc.tile_pool(name="vpow", bufs=1) as vpowp,
            tc.tile_pool(name="psum", bufs=2, space="PSUM") as psump,
        ):
            twb = wpool.tile([128, FATW], CT, name="twb")
            nc.sync.dma_start(twb[:], wb1.ap())
            twjs = {}
            tb0 = {}

            def load_value_weights(cv):
                wjt = {1: wj1, 2: wj2}[cv]
                bt = {1: b01, 2: b02}[cv]
                for j in range(8):
                    for ck in range(3):
                        t = wpool.tile([96, 32], DT.float32,
                                       name=f"wj{cv}_{j}_{ck}")
                        nc.sync.dma_start(
                            t[:], wjt.ap()[j, ck * 96:(ck + 1) * 96, :])
                        twjs[(cv, j, ck)] = t
                t = wpool.tile([32, 1], DT.float32, name=f"b0_{cv}")
                nc.sync.dma_start(t[:], bt.ap())
                tb0[cv] = t
            zcol = constp.tile([128, 1], DT.float32, name="zcol")
            nc.gpsimd.memset(zcol[:], 0.0)
            hcol = constp.tile([128, 1], DT.float32, name="hcol")
            nc.gpsimd.memset(hcol[:], h0)
            zfill = constp.tile([128, 153], CT, name="zfill")
            nc.gpsimd.memset(zfill[:], 0.0)
            zfill32 = constp.tile([128, 153], DT.float32, name="zfill32")
            nc.gpsimd.memset(zfill32[:], 0.0)
            for _repidx in range(repeat):
                for t in (cm, ch):
                    nc.sync.dma_start(
                        AP(tensor=t, offset=0, ap=[[1, ROWS1 * 34 * 32]]),
                        zfill[:])
                nc.sync.dma_start(
                    AP(tensor=cvv, offset=0, ap=[[1, 32 * ROWS1 * 34]]),
                    zfill32[:])

                sttscr = constp.tile([128, CKK], CT, name="sttscr")

                def _sl(t, o):
                    return t[:, o * CKK:(o + 1) * CKK]

                def _as3(t):
                    return AP(tensor=t.tensor, offset=t.offset,
                              ap=[list(t.ap[0]), [CKK, 32], [1, CKK]])

                def bounds_head(pw, pm):
                    """a = pm - w; b = |a| — issued one tile ahead."""
                    a = fatp.tile([128, FATW], CT, name="fatA",
                                  tag="fatA")[:pw]
                    b = fatp.tile([128, FATW], CT, name="fatB",
                                  tag="fatB")[:pw]
                    nc.vector.tensor_tensor(_as3(a), _rep(pm, 32),
                                            _as3(twb[:pw]), A.subtract)
                    nc.scalar.activation(b[:], a[:], F.Abs)
                    return (b,)

                def bounds_tail(conv, pw, btl, ph, lm):
                    (b,) = btl
                    c = fatp.tile([128, FATW], CT, name="fatC", tag="fatC",
                                  bufs=1)
                    d = fatp.tile([128, FATW], CT, name="fatD", tag="fatD",
                                  bufs=1)
                    e = fatp.tile([128, FATW], CT, name="fatE", tag="fatE",
                                  bufs=1)
                    zl = smallp.tile([128, 32], DT.float32, name="zl", tag="zl")
                    zu = smallp.tile([128, 32], DT.float32, name="zu", tag="zu")
                    c, d, e = c[:pw], d[:pw], e[:pw]
                    zl, zu = zl[:pw], zu[:pw]
                    scr = sttscr[:pw]
                    sl = _sl
                    as3 = _as3
                    if conv == 1:
                        # q = relu(b - h0) in ONE fused TS; s2 = (b + h0)^2
                        # in ONE ACT Square with +h0 bias
                        nc.vector.tensor_scalar(c[:], b[:], h0, 0.0,
                                                A.subtract, A.max)
                        nc.scalar.activation(e[:], c[:], F.Square)      # r2
                        nc.vector.tensor_tensor(c[:], e[:], e[:], A.mult)  # r4
                        nc.scalar.activation(e[:], b[:], F.Square,
                                             bias=hcol[:pw])            # s2
                        nc.vector.tensor_tensor(d[:], e[:], e[:], A.mult)  # s4
                    else:
                        nc.vector.tensor_tensor(as3(c), as3(b), _rep(ph, 32),
                                                A.subtract)             # q
                        nc.vector.tensor_tensor(as3(d), as3(b), _rep(ph, 32),
                                                A.add)                  # s
                        nc.vector.tensor_scalar(c[:], c[:], 0.0, None, A.max)
                        nc.scalar.activation(e[:], c[:], F.Square)      # r2
                        nc.scalar.activation(c[:], e[:], F.Square)      # r4
                        nc.scalar.activation(e[:], d[:], F.Square)      # s2
                        nc.scalar.activation(d[:], e[:], F.Square)      # s4
                    rsum, ssum = c, d
                    for o in range(32):
                        nc.vector.scalar_tensor_tensor(
                            scr[:], sl(rsum, o), 0.0, sl(rsum, o), A.add, A.mult,
                            accum_out=zl[:, o:o + 1])
                    for o in range(32):
                        nc.vector.scalar_tensor_tensor(
                            scr[:], sl(ssum, o), 0.0, sl(ssum, o), A.add, A.mult,
                            accum_out=zu[:, o:o + 1])
                    rdt = CT if conv == 1 else DT.float32
                    bias = lm if conv == 1 else zcol[:pw]
                    roots = []
                    for z in (zl, zu):
                        lnz = smallp.tile([128, 32], DT.float32, name="lnz",
                                          tag="lnz")[:pw]
                        rt = smallp.tile([128, 32], rdt, name=f"rt{conv}",
                                         tag=f"rt{conv}")[:pw]
                        nc.scalar.activation(lnz[:], z[:], F.Ln)
                        nc.scalar.activation(rt[:], lnz[:], F.Exp, bias=bias[:],
                                             scale=0.125)
                        roots.append(rt)
                    return roots

                def value_conv(conv, src_dram, hw, wtile_key, mask):
                    nrows = hw // 32
                    px = []
                    for dy in range(3):
                        t = vpowp.tile([96, hw], DT.float32, name=f"px{dy}",
                                       tag=f"px{dy}")
                        for dx in range(3):
                            src = AP(tensor=src_dram,
                                     offset=dy * 34 + dx,
                                     ap=[[20 * 34 if conv == 1 else ROWS1 * 34, 32],
                                         [34, nrows], [1, 32]])
                            nc.sync.dma_start(t[dx * 32:(dx + 1) * 32, :], src)
                        px.append(t)
                    nps = (hw + 511) // 512
                    psums = [psump.tile([32, min(512, hw - i * 512)], DT.float32,
                                        name=f"vps{i}", tag=f"vps{i}")
                             for i in range(nps)]

                    def mm(j, ck, t, start):
                        for i, ps in enumerate(psums):
                            nc.tensor.matmul(
                                ps[:], twjs[(conv, j, ck)][:],
                                t[:, i * 512:i * 512 + ps.shape[1]],
                                start=start, stop=(j == 7))
                    for ck in range(3):
                        p1 = px[ck]
                        p2 = vpowp.tile([96, hw], DT.float32, name="p2", tag="p2")
                        p4 = vpowp.tile([96, hw], DT.float32, name="p4", tag="p4")
                        tmp = vpowp.tile([96, hw], DT.float32, name="tmp",
                                         tag="tmp")
                        mm(0, ck, p1, start=(ck == 0))
                        nc.vector.tensor_tensor(p2[:], p1[:], p1[:], A.mult)
                        mm(1, ck, p2, start=False)
                        nc.vector.tensor_tensor(tmp[:], p2[:], p1[:], A.mult)
                        mm(2, ck, tmp, start=False)
                        nc.vector.tensor_tensor(p4[:], p2[:], p2[:], A.mult)
                        mm(3, ck, p4, start=False)
                        nc.vector.tensor_tensor(tmp[:], p4[:], p1[:], A.mult)
                        mm(4, ck, tmp, start=False)
                        nc.vector.tensor_tensor(tmp[:], p4[:], p2[:], A.mult)
                        mm(5, ck, tmp, start=False)
                        nc.vector.tensor_tensor(tmp[:], tmp[:], p1[:], A.mult)
                        mm(6, ck, tmp, start=False)
                        nc.vector.tensor_tensor(tmp[:], p4[:], p4[:], A.mult)
                        mm(7, ck, tmp, start=False)
                    y = smallp.tile([32, HW1], DT.float32, name=f"yv{conv}",
                                    tag=f"yv{conv}")[:, :hw]
                    for i, ps in enumerate(psums):
                        w = ps.shape[1]
                        seg = y[:, i * 512:i * 512 + w]
                        nc.scalar.activation(seg, ps[:], F.Relu)
                        nc.scalar.activation(seg, seg, F.Ln, bias=tb0[conv][:])
                        nc.scalar.activation(seg, seg, F.Exp, scale=0.125)
                    if mask is not None:
                        nc.vector.tensor_tensor(y[:], y[:], mask, A.mult)
                    return y

                # ========== conv1 (one-tile-lookahead pipeline) ==========
                y1v = None
                tiles1 = _hwtiles(HW1)
                heads = {}
                for ti in range(len(tiles1) + 1):
                    if ti < len(tiles1):
                        p0, pw = tiles1[ti]
                        pm = patchp.tile([128, CKK], CT, name="pm1",
                                         tag="pm1")[:pw]
                        _dma_patch(nc, pm, mp, p0 // 32, pw // 32)
                        heads[ti] = (p0, pw, bounds_head(pw, pm))
                    if ti == 1:
                        load_value_weights(1)
                        y1v = value_conv(1, xpc, HW1, 1, None)
                        vm = smallp.tile([32, HW1], DT.float32, name="vm",
                                         tag="vm")
                        nc.sync.dma_start(vm[:], vmask.ap())
                        nc.vector.tensor_tensor(y1v[:], y1v[:], vm[:], A.mult)
                        nc.sync.dma_start(
                            AP(tensor=cvv, offset=1,
                               ap=[[ROWS1 * 34, 32], [34, ROWS1], [1, 32]]),
                            y1v[:])
                    if ti - 1 not in heads:
                        continue
                    p0, pw, btl = heads.pop(ti - 1)
                    y0 = p0 // 32
                    nrows = pw // 32
                    lm = smallp.tile([128, 1], DT.float32, name="lm",
                                     tag="lm")[:pw]
                    nc.sync.dma_start(lm[:], lmask.ap()[p0:p0 + pw, :])
                    dl1, du1 = bounds_tail(1, pw, btl, None, lm)
                    m2 = smallp.tile([128, 32], CT, name="m2", tag="m2")[:pw]
                    h2 = smallp.tile([128, 32], CT, name="h2", tag="h2")[:pw]
                    nc.vector.tensor_tensor(m2[:], dl1[:], du1[:], A.add)
                    nc.vector.tensor_scalar(m2[:], m2[:], 0.5, None, A.mult)
                    nc.vector.tensor_tensor(h2[:], du1[:], dl1[:], A.subtract)
                    nc.vector.tensor_scalar(h2[:], h2[:], 0.5, None, A.mult)
                    nc.sync.dma_start(_canvas_interior(cm, y0, nrows), m2[:])
                    nc.sync.dma_start(_canvas_interior(ch, y0, nrows), h2[:])
                nc.sync.dma_start(twb[:], wb2.ap())

                # ================= conv2 =================
                load_value_weights(2)
                y2v = value_conv(2, cvv, HW2, 2, None)
                xcct = smallp.tile([32, HW2], DT.float32, name="xcct", tag="xcct")
                nc.sync.dma_start(xcct[:], xcc.ap())
                nc.vector.tensor_tensor(y2v[:], y2v[:], xcct[:], A.add)
                nc.scalar.activation(y2v[:], y2v[:], F.Relu)
                nc.sync.dma_start(out_v.ap(), y2v[:])
                tiles2 = _hwtiles(HW2)
                heads = {}
                for ti in range(len(tiles2) + 1):
                    if ti < len(tiles2):
                        p0, pw = tiles2[ti]
                        pmid = patchp.tile([128, CKK], CT, name="pmid",
                                           tag="pmid")[:pw]
                        phh = patchp.tile([128, CKK], CT, name="phh",
                                          tag="phh")[:pw]
                        _dma_patch(nc, pmid, cm, p0 // 32, pw // 32)
                        _dma_patch(nc, phh, ch, p0 // 32, pw // 32)
                        heads[ti] = (p0, pw, bounds_head(pw, pmid), phh)
                    if ti - 1 not in heads:
                        continue
                    p0, pw, btl, phh = heads.pop(ti - 1)
                    dl2, du2 = bounds_tail(2, pw, btl, phh, None)
                    lct = smallp.tile([128, 32], DT.float32, name="lct",
                                      tag="lct")[:pw]
                    uct = smallp.tile([128, 32], DT.float32, name="uct",
                                      tag="uct")[:pw]
                    nc.sync.dma_start(lct[:], lch.ap()[p0:p0 + pw, :])
                    nc.sync.dma_start(uct[:], uch.ap()[p0:p0 + pw, :])
                    for k, (rt, resid) in enumerate(((dl2, lct), (du2, uct))):
                        ro = smallp.tile([128, 32], DT.float32, name="ro",
                                         tag="ro")[:pw]
                        nc.vector.tensor_tensor(ro[:], rt[:], resid[:], A.add)
                        nc.scalar.activation(ro[:], ro[:], F.Relu)
                        nc.sync.dma_start(out_b.ap()[k, p0:p0 + pw, :], ro[:])
    return nc


_CACHE = {}


def _get_nc(repeat=1):
    key = f"nc{repeat}"
    if key not in _CACHE:
        _CACHE[key] = _build(repeat)
    return _CACHE[key]


def _norm_w(w):
    """[32,32,3,3] -> [32,288] mean-normalized, (dy,dx,c)-ordered."""
    wf = w.reshape(32, -1).astype(np.float32)
    wf = wf - wf.mean(axis=1, keepdims=True)
    return np.ascontiguousarray(
        wf.reshape(32, 32, 3, 3).transpose(0, 2, 3, 1).reshape(32, 288))


def _w_expand(wn):
    """[32,288] -> [128, 32*288] partition-broadcast, CT."""
    row = wn.reshape(1, 32 * 288)
    return np.ascontiguousarray(
        np.broadcast_to(row, (128, 32 * 288))).astype(NPCT)


def _prep_in_maps(x, weight1, weight2, lower=None, upper=None):
    x = np.asarray(x, np.float32)
    global _PREP_LU
    _PREP_LU = (np.asarray(lower, np.float32) if lower is not None else x - EPS,
                np.asarray(upper, np.float32) if upper is not None else x + EPS)
    wn1 = _norm_w(np.asarray(weight1, np.float32))
    wn2 = _norm_w(np.asarray(weight2, np.float32))
    w1 = _w_expand(wn1)
    w2 = _w_expand(wn2)
    from math import comb
    def wj_of(wn):
        # wj[j-1][k, o] = C(8,j) * (-w[o,k])^(8-j), j = 1..8
        out = np.zeros((8, CKK, 32), np.float32)
        for j in range(1, 9):
            out[j - 1] = (comb(8, j) * (-wn.T) ** (8 - j)).astype(np.float32)
        return out
    wj1 = wj_of(wn1)
    wj2 = wj_of(wn2)
    b01 = (wn1.astype(np.float64) ** 8).sum(1).astype(np.float32).reshape(32, 1)
    b02 = (wn2.astype(np.float64) ** 8).sum(1).astype(np.float32).reshape(32, 1)

    in_maps = []
    lo, up = _PREP_LU
    m1 = (lo + up) * 0.5
    h1 = (up - lo) * 0.5
    for core in range(8):
        b, half = core // 2, core % 2
        r0 = half * 16
        mp = np.zeros((20, 34, 32), np.float32)
        hpav = np.zeros((20, 34, 32), np.float32)
        xpcc = np.zeros((32, 20, 34), np.float32)
        for i in range(20):
            a = r0 - 2 + i
            if 0 <= a < H:
                mp[i, 1:33, :] = m1[b, :, a, :].T
                hpav[i, 1:33, :] = h1[b, :, a, :].T
                xpcc[:, i, 1:33] = x[b, :, a, :]
        lch = np.ascontiguousarray(
            lo[b, :, r0:r0 + 16, :].transpose(1, 2, 0).reshape(HW2, 32))
        uch = np.ascontiguousarray(
            up[b, :, r0:r0 + 16, :].transpose(1, 2, 0).reshape(HW2, 32))
        xcc = np.ascontiguousarray(
            x[b, :, r0:r0 + 16, :].reshape(32, HW2))
        lm = np.zeros((HW1, 1), np.float32)
        vm = np.ones((32, HW1), np.float32)
        if half == 0:
            lm[:32] = NEGINF
            vm[:, :32] = 0.0
        else:
            lm[-32:] = NEGINF
            vm[:, -32:] = 0.0
        in_maps.append({
            "mp": mp.astype(NPCT), "hp": hpav.astype(NPCT), "xpc": xpcc,
            "lch": lch, "uch": uch, "xcc": xcc,
            "lmask": lm, "vmask": vm,
            "wb1": w1, "wb2": w2, "wj1": wj1, "wj2": wj2,
            "b01": b01, "b02": b02,
        })
    return in_maps


def _unshard(results):
    full = np.zeros((3, B, C, H, W), np.float32)
    for core in range(8):
        b, half = core // 2, core % 2
        r0 = half * 16
        ob = results[core]["out_b"]           # [2, 512, 32] (hw, c)
        ov = results[core]["out_v"]           # [32, 512]    (c, hw)
        full[0, b, :, r0:r0 + 16, :] = ov.reshape(32, 16, 32)
        full[1:, b, :, r0:r0 + 16, :] = (
            ob.reshape(2, 16, 32, 32).transpose(0, 3, 1, 2))
    return full


def _get_nc_u2(h0, repeat=1):
    key = f"ncu2_{repeat}_{h0:.9e}"
    if key not in _CACHE:
        _CACHE[key] = _build_u2(h0, repeat)
    return _CACHE[key]


def _prep_in_maps_u2(x, weight1, weight2, lower, upper):
    maps = _prep_in_maps(x, weight1, weight2, lower, upper)
    for m in maps:
        del m["hp"]
    return maps


def kernel(x, lower, upper, weight1, weight2):
    lo = np.asarray(lower, np.float32)
    up = np.asarray(upper, np.float32)
    hv = (up - lo) * 0.5
    h0 = float(np.median(hv))
    uniform = (h0 > 0 and
               float(np.max(np.abs(hv - h0))) <= max(1e-5, 1e-3 * h0))
    if uniform:
        in_maps = _prep_in_maps_u2(x, weight1, weight2, lo, up)
        nc = _get_nc_u2(h0)
    else:
        in_maps = _prep_in_maps(x, weight1, weight2, lo, up)
        nc = _get_nc()
    res = run_bass_kernel_spmd(nc, in_maps, list(range(8)))
    _CACHE["last_results"] = res
    return _unshard(res.results)



# revision 5
# speedup vs baseline: 1.8873x; 1.8873x over previous
"""Lp-distance (p=8) BasicBlock kernel for 8 Trainium2 NeuronCores.

Math (per conv, per output channel o), with mid=(pl+pu)/2, h=(pu-pl)/2 and
t = |w - mid| per patch element:
    value = (sum_ckk (patch_x - w[o])^8)^(1/8)          (binomial, on PE)
    dl    = (sum_ckk relu(t - h)^8)^(1/8)               (fp16 chains, DVE+ACT)
    du    = (sum_ckk (t + h)^8)^(1/8)
The first bound_relu is an exact no-op (all conv outputs are >= 0), so conv2
consumes conv1 outputs directly.

Sharding: (batch=4) x (H-halves=2) -> 8 cores, zero collectives.  Each core
gets host-padded mid/h/x slabs with a 2-row halo, computes conv1 on 18 rows
(one fictional edge row zeroed via the Lp-root's exp bias = -1e30 mask),
bounces conv1 results through DRAM canvases, computes conv2 on 16 rows, adds
the residual (actual lower/upper inputs) and final relu.

Bounds layout: hw-positions on partitions, (o, ckk) on the free dim; heavy
chains are fat [pw, 32*288] fp16 passes (DVE tensor_tensor at 2x, ACT Square)
with per-o sums via scalar_tensor_tensor's fused fp32 accumulator; the weight
operand broadcasts via a step-0 AP dim.  Value path: fp32 binomial expansion
sum_j C(8,j) px^j (-w)^(8-j) as 24 accumulating matmuls on the otherwise-idle
TensorE in (c, hw) layout, j=0 term folded into the Ln bias.  Lp root =
exp(ln(z)/8) on ACT; Ln/Exp/Square/Abs/Relu all live in one ACT table set.

Uniform-h fast path (_build_u2): when upper-lower is a uniform 2*h0 (the
harness always generates lower=x-eps, upper=x+eps), conv1's h is baked as an
immediate: no hp canvas / h-patch DMAs, q-chain starts with one fused
tensor_scalar (b-h0 then relu), s-chain with one ACT Square(b + h0*bias).
kernel() detects uniformity at runtime and falls back to the general build
otherwise.

Toolchain notes: this walrus build allows at most one sync-wait per
instruction (see _split_multiwait) and supports no custom-DVE ops; AluOpType
pow/abs_max fail codegen; tensor_scalar two-op fusions and ACT bias tiles
are safe.
"""
import json

import ml_dtypes
import numpy as np

import concourse.bass as bass
import concourse.bass2jax as bass2jax
import concourse.bass_utils as bass_utils
import concourse.mybir as mybir
import concourse.tile as tile
from concourse.bass import AP
from concourse.bass_utils import run_bass_kernel_spmd

# ---------------------------------------------------------------------------
# Walrus workaround: this toolchain's codegen accepts at most ONE sync-wait
# per instruction; Tile emits several on drains/joins.  Split the extras onto
# preceding same-engine NoOps (semantically identical: waits run in order).
_orig_cbk = bass_utils.compile_bir_kernel


def _split_multiwait(bir_bytes):
    bir = json.loads(bir_bytes)
    ctr = 0
    for f in bir.get("functions", []):
        for blk in f.get("blocks", []):
            out = []
            for ins in blk["instructions"]:
                si = ins.get("sync_info")
                ow = (si or {}).get("on_wait") or []
                if len(ow) > 1:
                    si["on_wait"] = ow[-1:]
                    for w in ow[:-1]:
                        ctr += 1
                        out.append({
                            "debug": ins.get("debug", 0),
                            "engine": ins["engine"], "ins": [],
                            "name": f"I-WSPLIT{ctr}", "opcode": "NoOp",
                            "outs": [],
                            "sync_info": {"on_wait": [w], "on_update": []}})
                out.append(ins)
            blk["instructions"][:] = out
    return json.dumps(bir).encode()


def _patched_cbk(bir_json, tmpdir, neff_name="file.neff"):
    return _orig_cbk(_split_multiwait(bir_json), tmpdir, neff_name)


if bass_utils.compile_bir_kernel is not _patched_cbk:
    bass_utils.compile_bir_kernel = _patched_cbk
    bass2jax.compile_bir_kernel = _patched_cbk

# ---------------------------------------------------------------------------
F = mybir.ActivationFunctionType
A = mybir.AluOpType
DT = mybir.dt

CT = DT.float16             # compute dtype: same DVE speed tier as bf16, 8x finer
NPCT = np.float16
OG = 32                     # output channels per fat pass (fat width = OG*288)
EPS = 0.1
NEGINF = -1e30              # exp(x + NEGINF) == 0 in fp32

B, C, H, W = 4, 32, 32, 32
CKK = 288                   # 3*3*32, ordered (dy, dx, c)
ROWS1 = 18                  # conv1 output rows per core (incl 1 fictional)
ROWS2 = 16                  # conv2 output rows per core
HW1 = ROWS1 * 32            # 576
HW2 = ROWS2 * 32            # 512
CV_ROWSTRIDE = 34 * 32      # canvas row stride in elements


def _hwtiles(hw):
    """[(p0, pw), ...] partition tiles covering hw positions."""
    out = []
    p = 0
    while p < hw:
        pw = min(128, hw - p)
        out.append((p, pw))
        p += pw
    return out


def _patch_src_dy(t, y0, nrows, dy):
    """Overlapping 3x3 patch gather, one dy slice: out position (y,x) reads
    row y0+y+dy, cols x..x+2, all c (free order (dx, c), contiguous 96)."""
    return AP(tensor=t, offset=(y0 + dy) * CV_ROWSTRIDE,
              ap=[[CV_ROWSTRIDE, nrows], [32, 32], [1, 96]])


def _dma_patch(nc, dst, src_t, y0, nrows):
    for dy in range(3):
        nc.sync.dma_start(dst[:, dy * 96:(dy + 1) * 96],
                          _patch_src_dy(src_t, y0, nrows, dy))


def _canvas_interior(t, y0, nrows):
    """Write [nrows*32, 32] (hw, c) into canvas rows y0.., cols 1..32."""
    return AP(tensor=t, offset=(y0 * 34 + 1) * 32,
              ap=[[CV_ROWSTRIDE, nrows], [32, 32], [1, 32]])


def _rep(ap_, n):
    """Repeat a [P, W] AP n times along a step-0 middle dim -> [P, n, W]."""
    return AP(tensor=ap_.tensor, offset=ap_.offset,
              ap=[list(ap_.ap[0]), [0, n], list(ap_.ap[-1])])


def _build(repeat=1):
    nc = bass.Bass("TRN2", target_bir_lowering=False, debug=False,
                   num_devices=8)
    mp = nc.dram_tensor("mp", [20, 34, 32], CT, kind="ExternalInput")
    hp = nc.dram_tensor("hp", [20, 34, 32], CT, kind="ExternalInput")
    xpc = nc.dram_tensor("xpc", [32, 20, 34], DT.float32, kind="ExternalInput")
    lch = nc.dram_tensor("lch", [HW2, 32], DT.float32, kind="ExternalInput")
    uch = nc.dram_tensor("uch", [HW2, 32], DT.float32, kind="ExternalInput")
    xcc = nc.dram_tensor("xcc", [32, HW2], DT.float32, kind="ExternalInput")
    lmask = nc.dram_tensor("lmask", [HW1, 1], DT.float32, kind="ExternalInput")
    vmask = nc.dram_tensor("vmask", [32, HW1], DT.float32, kind="ExternalInput")
    wb1 = nc.dram_tensor("wb1", [128, 32 * CKK], CT, kind="ExternalInput")
    wb2 = nc.dram_tensor("wb2", [128, 32 * CKK], CT, kind="ExternalInput")
    wj1 = nc.dram_tensor("wj1", [8, CKK, 32], DT.float32, kind="ExternalInput")
    wj2 = nc.dram_tensor("wj2", [8, CKK, 32], DT.float32, kind="ExternalInput")
    b01 = nc.dram_tensor("b01", [32, 1], DT.float32, kind="ExternalInput")
    b02 = nc.dram_tensor("b02", [32, 1], DT.float32, kind="ExternalInput")
    cm = nc.dram_tensor("cm", [ROWS1, 34, 32], CT)
    ch = nc.dram_tensor("ch", [ROWS1, 34, 32], CT)
    cvv = nc.dram_tensor("cvv", [32, ROWS1, 34], DT.float32)
    out_b = nc.dram_tensor("out_b", [2, HW2, 32], DT.float32,
                           kind="ExternalOutput")
    out_v = nc.dram_tensor("out_v", [32, HW2], DT.float32,
                           kind="ExternalOutput")

    FATW = 32 * CKK

    with tile.TileContext(nc) as tc:
        with (
            tc.tile_pool(name="const", bufs=1) as constp,
            tc.tile_pool(name="wpool", bufs=1) as wpool,
            tc.tile_pool(name="patch", bufs=2) as patchp,
            tc.tile_pool(name="fat", bufs=2) as fatp,
            tc.tile_pool(name="small", bufs=3) as smallp,
            tc.tile_pool(name="vpow", bufs=1) as vpowp,
            tc.tile_pool(name="psum", bufs=2, space="PSUM") as psump,
        ):
            twb = wpool.tile([128, FATW], CT, name="twb")
            nc.sync.dma_start(twb[:], wb1.ap())
            # value lhsT chunks loaded lazily at first use (keeps startup
            # DMA on the bounds-critical path)
            twjs = {}
            tb0 = {}

            def load_value_weights(cv):
                wjt = {1: wj1, 2: wj2}[cv]
                bt = {1: b01, 2: b02}[cv]
                for j in range(8):
                    for ck in range(3):
                        t = wpool.tile([96, 32], DT.float32,
                                       name=f"wj{cv}_{j}_{ck}")
                        nc.sync.dma_start(
                            t[:], wjt.ap()[j, ck * 96:(ck + 1) * 96, :])
                        twjs[(cv, j, ck)] = t
                t = wpool.tile([32, 1], DT.float32, name=f"b0_{cv}")
                nc.sync.dma_start(t[:], bt.ap())
                tb0[cv] = t
            zcol = constp.tile([128, 1], DT.float32, name="zcol")
            nc.gpsimd.memset(zcol[:], 0.0)
            nepscol = constp.tile([128, 1], DT.float32, name="nepscol")
            nc.gpsimd.memset(nepscol[:], -EPS)
            pepscol = constp.tile([128, 1], DT.float32, name="pepscol")
            nc.gpsimd.memset(pepscol[:], EPS)
            zfill = constp.tile([128, 153], CT, name="zfill")
            nc.gpsimd.memset(zfill[:], 0.0)
            zfill32 = constp.tile([128, 153], DT.float32, name="zfill32")
            nc.gpsimd.memset(zfill32[:], 0.0)
            for _repidx in range(repeat):
                for t in (cm, ch):
                    nc.sync.dma_start(
                        AP(tensor=t, offset=0, ap=[[1, ROWS1 * 34 * 32]]),
                        zfill[:])
                # exact-count zero fill for cvv: 32*18*34 = 19584 = 128*153
                nc.sync.dma_start(
                    AP(tensor=cvv, offset=0, ap=[[1, 32 * ROWS1 * 34]]),
                    zfill32[:])

                sttscr = constp.tile([128, CKK], CT, name="sttscr")

                def bounds_tile(conv, p0, pw, wtile, pm, ph, lm):
                    """Bounds chains for one hw-tile; returns (dl_rt, du_rt)."""
                    a = fatp.tile([128, FATW], CT, name="fatA", tag="fatA")
                    b = fatp.tile([128, FATW], CT, name="fatB", tag="fatB")
                    c = fatp.tile([128, FATW], CT, name="fatC", tag="fatC",
                                  bufs=1)
                    d = fatp.tile([128, FATW], CT, name="fatD", tag="fatD",
                                  bufs=1)  # bufs kept at 1: SBUF-bound
                    e = fatp.tile([128, FATW], CT, name="fatE", tag="fatE",
                                  bufs=1)
                    zl = smallp.tile([128, 32], DT.float32, name="zl", tag="zl")
                    zu = smallp.tile([128, 32], DT.float32, name="zu", tag="zu")
                    a, b, c, d, e = a[:pw], b[:pw], c[:pw], d[:pw], e[:pw]
                    zl, zu = zl[:pw], zu[:pw]
                    scr = sttscr[:pw]

                    def sl(t, o):
                        return t[:, o * CKK:(o + 1) * CKK]

                    def as3(t):
                        return AP(tensor=t.tensor, offset=t.offset,
                                  ap=[list(t.ap[0]), [CKK, 32], [1, CKK]])

                    # t = pm(rep) - w ; at = |t|
                    nc.vector.tensor_tensor(as3(a), _rep(pm, 32),
                                            as3(wtile[:pw]), A.subtract)
                    nc.scalar.activation(b[:], a[:], F.Abs)
                    # chains (tensor h for both convs)
                    nc.vector.tensor_tensor(as3(c), as3(b), _rep(ph, 32),
                                            A.subtract)             # q
                    nc.vector.tensor_tensor(as3(d), as3(b), _rep(ph, 32),
                                            A.add)                  # s
                    nc.vector.tensor_scalar(c[:], c[:], 0.0, None, A.max)
                    nc.scalar.activation(e[:], c[:], F.Square)      # r2
                    nc.scalar.activation(c[:], e[:], F.Square)      # r4
                    nc.scalar.activation(e[:], d[:], F.Square)      # s2
                    if conv == 1:
                        nc.vector.tensor_tensor(d[:], e[:], e[:], A.mult)
                    else:
                        nc.scalar.activation(d[:], e[:], F.Square)  # s4
                    rsum, ssum = c, d
                    for o in range(32):
                        nc.vector.scalar_tensor_tensor(
                            scr[:], sl(rsum, o), 0.0, sl(rsum, o), A.add, A.mult,
                            accum_out=zl[:, o:o + 1])
                    for o in range(32):
                        nc.vector.scalar_tensor_tensor(
                            scr[:], sl(ssum, o), 0.0, sl(ssum, o), A.add, A.mult,
                            accum_out=zu[:, o:o + 1])
                    rdt = CT if conv == 1 else DT.float32
                    bias = lm if conv == 1 else zcol[:pw]
                    roots = []
                    for z in (zl, zu):
                        lnz = smallp.tile([128, 32], DT.float32, name="lnz",
                                          tag="lnz")[:pw]
                        rt = smallp.tile([128, 32], rdt, name=f"rt{conv}",
                                         tag=f"rt{conv}")[:pw]
                        nc.scalar.activation(lnz[:], z[:], F.Ln)
                        nc.scalar.activation(rt[:], lnz[:], F.Exp, bias=bias[:],
                                             scale=0.125)
                        roots.append(rt)
                    return roots

                def value_conv(conv, src_dram, hw, wtile_key, mask):
                    """Binomial value path in (c, hw) layout via PE.
                    Returns y [32, hw] f32 SBUF tile (masked for conv1)."""
                    nrows = hw // 32
                    # patch chunks [96, hw] x3 (dy), rows (dx, c) ordered... NOTE:
                    # chunk rows must match wj ordering (dy, dx, c) c-minor.
                    px = []
                    for dy in range(3):
                        t = vpowp.tile([96, hw], DT.float32, name=f"px{dy}",
                                       tag=f"px{dy}")
                        for dx in range(3):
                            src = AP(tensor=src_dram,
                                     offset=dy * 34 + dx,
                                     ap=[[20 * 34 if conv == 1 else ROWS1 * 34, 32],
                                         [34, nrows], [1, 32]])
                            nc.sync.dma_start(t[dx * 32:(dx + 1) * 32, :], src)
                        px.append(t)
                    nps = (hw + 511) // 512
                    psums = [psump.tile([32, min(512, hw - i * 512)], DT.float32,
                                        name=f"vps{i}", tag=f"vps{i}")
                             for i in range(nps)]

                    def mm(j, ck, t, start):
                        for i, ps in enumerate(psums):
                            nc.tensor.matmul(
                                ps[:], twjs[(conv, j, ck)][:],
                                t[:, i * 512:i * 512 + ps.shape[1]],
                                start=start, stop=(j == 7))
                    for ck in range(3):
                        p1 = px[ck]
                        p2 = vpowp.tile([96, hw], DT.float32, name="p2", tag="p2")
                        p4 = vpowp.tile([96, hw], DT.float32, name="p4", tag="p4")
                        tmp = vpowp.tile([96, hw], DT.float32, name="tmp",
                                         tag="tmp")
                        mm(0, ck, p1, start=(ck == 0))          # j index 0 == x^1
                        nc.vector.tensor_tensor(p2[:], p1[:], p1[:], A.mult)
                        mm(1, ck, p2, start=False)
                        nc.vector.tensor_tensor(tmp[:], p2[:], p1[:], A.mult)
                        mm(2, ck, tmp, start=False)             # x^3
                        nc.vector.tensor_tensor(p4[:], p2[:], p2[:], A.mult)
                        mm(3, ck, p4, start=False)
                        nc.vector.tensor_tensor(tmp[:], p4[:], p1[:], A.mult)
                        mm(4, ck, tmp, start=False)             # x^5
                        nc.vector.tensor_tensor(tmp[:], p4[:], p2[:], A.mult)
                        mm(5, ck, tmp, start=False)             # x^6
                        nc.vector.tensor_tensor(tmp[:], tmp[:], p1[:], A.mult)
                        mm(6, ck, tmp, start=False)             # x^7
                        nc.vector.tensor_tensor(tmp[:], p4[:], p4[:], A.mult)
                        mm(7, ck, tmp, start=False)             # x^8
                    y = smallp.tile([32, HW1], DT.float32, name=f"yv{conv}",
                                    tag=f"yv{conv}")[:, :hw]
                    for i, ps in enumerate(psums):
                        w = ps.shape[1]
                        seg = y[:, i * 512:i * 512 + w]
                        nc.scalar.activation(seg, ps[:], F.Relu)
                        nc.scalar.activation(seg, seg, F.Ln, bias=tb0[conv][:])
                        nc.scalar.activation(seg, seg, F.Exp, scale=0.125)
                    if mask is not None:
                        nc.vector.tensor_tensor(y[:], y[:], mask, A.mult)
                    return y

                # ================= conv1 =================
                y1v = None
                for ti, (p0, pw) in enumerate(_hwtiles(HW1)):
                    y0 = p0 // 32
                    nrows = pw // 32
                    pm = patchp.tile([128, CKK], CT, name="pm1", tag="pm1")[:pw]
                    _dma_patch(nc, pm, mp, y0, nrows)
                    ph1 = patchp.tile([128, CKK], CT, name="ph1",
                                      tag="ph1")[:pw]
                    _dma_patch(nc, ph1, hp, y0, nrows)
                    lm = smallp.tile([128, 1], DT.float32, name="lm",
                                     tag="lm")[:pw]
                    nc.sync.dma_start(lm[:], lmask.ap()[p0:p0 + pw, :])
                    dl1, du1 = bounds_tile(1, p0, pw, twb, pm, ph1, lm)
                    m2 = smallp.tile([128, 32], CT, name="m2", tag="m2")[:pw]
                    h2 = smallp.tile([128, 32], CT, name="h2", tag="h2")[:pw]
                    nc.vector.tensor_tensor(m2[:], dl1[:], du1[:], A.add)
                    nc.vector.tensor_scalar(m2[:], m2[:], 0.5, None, A.mult)
                    nc.vector.tensor_tensor(h2[:], du1[:], dl1[:], A.subtract)
                    nc.vector.tensor_scalar(h2[:], h2[:], 0.5, None, A.mult)
                    nc.sync.dma_start(_canvas_interior(cm, y0, nrows), m2[:])
                    nc.sync.dma_start(_canvas_interior(ch, y0, nrows), h2[:])
                    if ti == 0:
                        # value path (c, hw) via PE, interleaves with bounds
                        load_value_weights(1)
                        y1v = value_conv(1, xpc, HW1, 1, None)
                        vm = smallp.tile([32, HW1], DT.float32, name="vm",
                                         tag="vm")
                        nc.sync.dma_start(vm[:], vmask.ap())
                        nc.vector.tensor_tensor(y1v[:], y1v[:], vm[:], A.mult)
                        nc.sync.dma_start(
                            AP(tensor=cvv, offset=1,
                               ap=[[ROWS1 * 34, 32], [34, ROWS1], [1, 32]]),
                            y1v[:])
                # reload shared weight tile for conv2
                nc.sync.dma_start(twb[:], wb2.ap())

                # ================= conv2 =================
                load_value_weights(2)
                y2v = value_conv(2, cvv, HW2, 2, None)
                xcct = smallp.tile([32, HW2], DT.float32, name="xcct", tag="xcct")
                nc.sync.dma_start(xcct[:], xcc.ap())
                nc.vector.tensor_tensor(y2v[:], y2v[:], xcct[:], A.add)
                nc.scalar.activation(y2v[:], y2v[:], F.Relu)
                nc.sync.dma_start(out_v.ap(), y2v[:])
                tiles2 = _hwtiles(HW2)
                heads = {}
                for ti in range(len(tiles2) + 1):
                    if ti < len(tiles2):
                        p0, pw = tiles2[ti]
                        pmid = patchp.tile([128, CKK], CT, name="pmid",
                                           tag="pmid")[:pw]
                        phh = patchp.tile([128, CKK], CT, name="phh",
                                          tag="phh")[:pw]
                        _dma_patch(nc, pmid, cm, p0 // 32, pw // 32)
                        _dma_patch(nc, phh, ch, p0 // 32, pw // 32)
                        heads[ti] = (p0, pw, bounds_head(pw, pmid), phh)
                    if ti - 1 not in heads:
                        continue
                    p0, pw, btl, phh = heads.pop(ti - 1)
                    dl2, du2 = bounds_tail(2, pw, btl, phh, None)
                    lct = smallp.tile([128, 32], DT.float32, name="lct",
                                      tag="lct")[:pw]
                    uct = smallp.tile([128, 32], DT.float32, name="uct",
                                      tag="uct")[:pw]
                    nc.sync.dma_start(lct[:], lch.ap()[p0:p0 + pw, :])
                    nc.sync.dma_start(uct[:], uch.ap()[p0:p0 + pw, :])
                    for k, (rt, resid) in enumerate(((dl2, lct), (du2, uct))):
                        ro = smallp.tile([128, 32], DT.float32, name="ro",
                                         tag="ro")[:pw]
                        nc.vector.tensor_tensor(ro[:], rt[:], resid[:], A.add)
                        nc.scalar.activation(ro[:], ro[:], F.Relu)
                        nc.sync.dma_start(out_b.ap()[k, p0:p0 + pw, :], ro[:])
    return nc



def _build_u2(h0, repeat=1):
    """Baseline structure with conv1 running on a scalar interval
    half-width h0: no hp canvas / ph patches; c-chain via one fused
    tensor_scalar (sub,max); s2 via ACT Square with +h0 bias."""
    nc = bass.Bass("TRN2", target_bir_lowering=False, debug=False,
                   num_devices=8)
    mp = nc.dram_tensor("mp", [20, 34, 32], CT, kind="ExternalInput")
    xpc = nc.dram_tensor("xpc", [32, 20, 34], DT.float32, kind="ExternalInput")
    lch = nc.dram_tensor("lch", [HW2, 32], DT.float32, kind="ExternalInput")
    uch = nc.dram_tensor("uch", [HW2, 32], DT.float32, kind="ExternalInput")
    xcc = nc.dram_tensor("xcc", [32, HW2], DT.float32, kind="ExternalInput")
    lmask = nc.dram_tensor("lmask", [HW1, 1], DT.float32, kind="ExternalInput")
    vmask = nc.dram_tensor("vmask", [32, HW1], DT.float32, kind="ExternalInput")
    wb1 = nc.dram_tensor("wb1", [128, 32 * CKK], CT, kind="ExternalInput")
    wb2 = nc.dram_tensor("wb2", [128, 32 * CKK], CT, kind="ExternalInput")
    wj1 = nc.dram_tensor("wj1", [8, CKK, 32], DT.float32, kind="ExternalInput")
    wj2 = nc.dram_tensor("wj2", [8, CKK, 32], DT.float32, kind="ExternalInput")
    b01 = nc.dram_tensor("b01", [32, 1], DT.float32, kind="ExternalInput")
    b02 = nc.dram_tensor("b02", [32, 1], DT.float32, kind="ExternalInput")
    cm = nc.dram_tensor("cm", [ROWS1, 34, 32], CT)
    ch = nc.dram_tensor("ch", [ROWS1, 34, 32], CT)
    cvv = nc.dram_tensor("cvv", [32, ROWS1, 34], DT.float32)
    out_b = nc.dram_tensor("out_b", [2, HW2, 32], DT.float32,
                           kind="ExternalOutput")
    out_v = nc.dram_tensor("out_v", [32, HW2], DT.float32,
                           kind="ExternalOutput")

    FATW = 32 * CKK

    with tile.TileContext(nc) as tc:
        with (
            tc.tile_pool(name="const", bufs=1) as constp,
            tc.tile_pool(name="wpool", bufs=1) as wpool,
            tc.tile_pool(name="patch", bufs=2) as patchp,
            tc.tile_pool(name="fat", bufs=2) as fatp,
            tc.tile_pool(name="small", bufs=3) as smallp,
            tc.tile_pool(name="vpow", bufs=1) as vpowp,
            tc.tile_pool(name="psum", bufs=2, space="PSUM") as psump,
        ):
            twb = wpool.tile([128, FATW], CT, name="twb")
            nc.sync.dma_start(twb[:], wb1.ap())
            twjs = {}
            tb0 = {}

            def load_value_weights(cv):
                wjt = {1: wj1, 2: wj2}[cv]
                bt = {1: b01, 2: b02}[cv]
                for j in range(8):
                    for ck in range(3):
                        t = wpool.tile([96, 32], DT.float32,
                                       name=f"wj{cv}_{j}_{ck}")
                        nc.sync.dma_start(
                            t[:], wjt.ap()[j, ck * 96:(ck + 1) * 96, :])
                        twjs[(cv, j, ck)] = t
                t = wpool.tile([32, 1], DT.float32, name=f"b0_{cv}")
                nc.sync.dma_start(t[:], bt.ap())
                tb0[cv] = t
            zcol = constp.tile([128, 1], DT.float32, name="zcol")
            nc.gpsimd.memset(zcol[:], 0.0)
            hcol = constp.tile([128, 1], DT.float32, name="hcol")
            nc.gpsimd.memset(hcol[:], h0)
            zfill = constp.tile([128, 153], CT, name="zfill")
            nc.gpsimd.memset(zfill[:], 0.0)
            zfill32 = constp.tile([128, 153], DT.float32, name="zfill32")
            nc.gpsimd.memset(zfill32[:], 0.0)
            for _repidx in range(repeat):
                for t in (cm, ch):
                    nc.sync.dma_start(
                        AP(tensor=t, offset=0, ap=[[1, ROWS1 * 34 * 32]]),
                        zfill[:])
                nc.sync.dma_start(
                    AP(tensor=cvv, offset=0, ap=[[1, 32 * ROWS1 * 34]]),
                    zfill32[:])

                sttscr = constp.tile([128, CKK], CT, name="sttscr")

                def _sl(t, o):
                    return t[:, o * CKK:(o + 1) * CKK]

                def _as3(t):
                    return AP(tensor=t.tensor, offset=t.offset,
                              ap=[list(t.ap[0]), [CKK, 32], [1, CKK]])

                def bounds_head(pw, pm):
                    """a = pm - w; b = |a| — issued one tile ahead."""
                    a = fatp.tile([128, FATW], CT, name="fatA",
                                  tag="fatA")[:pw]
                    b = fatp.tile([128, FATW], CT, name="fatB",
                                  tag="fatB")[:pw]
                    nc.vector.tensor_tensor(_as3(a), _rep(pm, 32),
                                            _as3(twb[:pw]), A.subtract)
                    nc.scalar.activation(b[:], a[:], F.Abs)
                    return (b,)

                def bounds_tail(conv, pw, btl, ph, lm):
                    (b,) = btl
                    c = fatp.tile([128, FATW], CT, name="fatC", tag="fatC",
                                  bufs=1)
                    d = fatp.tile([128, FATW], CT, name="fatD", tag="fatD",
                                  bufs=1)
                    e = fatp.tile([128, FATW], CT, name="fatE", tag="fatE",
                                  bufs=1)
                    zl = smallp.tile([128, 32], DT.float32, name="zl", tag="zl")
                    zu = smallp.tile([128, 32], DT.float32, name="zu", tag="zu")
                    c, d, e = c[:pw], d[:pw], e[:pw]
                    zl, zu = zl[:pw], zu[:pw]
                    scr = sttscr[:pw]
                    sl = _sl
                    as3 = _as3
                    if conv == 1:
                        # q = relu(b - h0) in ONE fused TS; s2 = (b + h0)^2
                        # in ONE ACT Square with +h0 bias
                        nc.vector.tensor_scalar(c[:], b[:], h0, 0.0,
                                                A.subtract, A.max)
                        nc.scalar.activation(e[:], c[:], F.Square)      # r2
                        nc.vector.tensor_tensor(c[:], e[:], e[:], A.mult)  # r4
                        nc.scalar.activation(e[:], b[:], F.Square,
                                             bias=hcol[:pw])            # s2
                        nc.vector.tensor_tensor(d[:], e[:], e[:], A.mult)  # s4
                    else:
                        nc.vector.tensor_tensor(as3(c), as3(b), _rep(ph, 32),
                                                A.subtract)             # q
                        nc.vector.tensor_tensor(as3(d), as3(b), _rep(ph, 32),
                                                A.add)                  # s
                        nc.vector.tensor_scalar(c[:], c[:], 0.0, None, A.max)
                        nc.scalar.activation(e[:], c[:], F.Square)      # r2
                        nc.scalar.activation(c[:], e[:], F.Square)      # r4
                        nc.scalar.activation(e[:], d[:], F.Square)      # s2
                        nc.scalar.activation(d[:], e[:], F.Square)      # s4
                    rsum, ssum = c, d
                    for o in range(32):
                        nc.vector.scalar_tensor_tensor(
                            scr[:], sl(rsum, o), 0.0, sl(rsum, o), A.add, A.mult,
                            accum_out=zl[:, o:o + 1])
                    for o in range(32):
                        nc.vector.scalar_tensor_tensor(
                            scr[:], sl(ssum, o), 0.0, sl(ssum, o), A.add, A.mult,
                            accum_out=zu[:, o:o + 1])
                    rdt = CT if conv == 1 else DT.float32
                    bias = lm if conv == 1 else zcol[:pw]
                    roots = []
                    for z in (zl, zu):
                        lnz = smallp.tile([128, 32], DT.float32, name="lnz",
                                          tag="lnz")[:pw]
                        rt = smallp.tile([128, 32], rdt, name=f"rt{conv}",
                                         tag=f"rt{conv}")[:pw]
                        nc.scalar.activation(lnz[:], z[:], F.Ln)
                        nc.scalar.activation(rt[:], lnz[:], F.Exp, bias=bias[:],
                                             scale=0.125)
                        roots.append(rt)
                    return roots

                def value_conv(conv, src_dram, hw, wtile_key, mask):
                    nrows = hw // 32
                    px = []
                    for dy in range(3):
                        t = vpowp.tile([96, hw], DT.float32, name=f"px{dy}",
                                       tag=f"px{dy}")
                        for dx in range(3):
                            src = AP(tensor=src_dram,
                                     offset=dy * 34 + dx,
                                     ap=[[20 * 34 if conv == 1 else ROWS1 * 34, 32],
                                         [34, nrows], [1, 32]])
                            nc.sync.dma_start(t[dx * 32:(dx + 1) * 32, :], src)
                        px.append(t)
                    nps = (hw + 511) // 512
                    psums = [psump.tile([32, min(512, hw - i * 512)], DT.float32,
                                        name=f"vps{i}", tag=f"vps{i}")
                             for i in range(nps)]

                    def mm(j, ck, t, start):
                        for i, ps in enumerate(psums):
                            nc.tensor.matmul(
                                ps[:], twjs[(conv, j, ck)][:],
                                t[:, i * 512:i * 512 + ps.shape[1]],
                                start=start, stop=(j == 7))
                    for ck in range(3):
                        p1 = px[ck]
                        p2 = vpowp.tile([96, hw], DT.float32, name="p2", tag="p2")
                        p4 = vpowp.tile([96, hw], DT.float32, name="p4", tag="p4")
                        tmp = vpowp.tile([96, hw], DT.float32, name="tmp",
                                         tag="tmp")
                        mm(0, ck, p1, start=(ck == 0))
                        nc.vector.tensor_tensor(p2[:], p1[:], p1[:], A.mult)
                        mm(1, ck, p2, start=False)
                        nc.vector.tensor_tensor(tmp[:], p2[:], p1[:], A.mult)
                        mm(2, ck, tmp, start=False)
                        nc.vector.tensor_tensor(p4[:], p2[:], p2[:], A.mult)
                        mm(3, ck, p4, start=False)
                        nc.vector.tensor_tensor(tmp[:], p4[:], p1[:], A.mult)
                        mm(4, ck, tmp, start=False)
                        nc.vector.tensor_tensor(tmp[:], p4[:], p2[:], A.mult)
                        mm(5, ck, tmp, start=False)
                        nc.vector.tensor_tensor(tmp[:], tmp[:], p1[:], A.mult)
                        mm(6, ck, tmp, start=False)
                        nc.vector.tensor_tensor(tmp[:], p4[:], p4[:], A.mult)
                        mm(7, ck, tmp, start=False)
                    y = smallp.tile([32, HW1], DT.float32, name=f"yv{conv}",
                                    tag=f"yv{conv}")[:, :hw]
                    for i, ps in enumerate(psums):
                        w = ps.shape[1]
                        seg = y[:, i * 512:i * 512 + w]
                        nc.scalar.activation(seg, ps[:], F.Relu)
                        nc.scalar.activation(seg, seg, F.Ln, bias=tb0[conv][:])
                        nc.scalar.activation(seg, seg, F.Exp, scale=0.125)
                    if mask is not None:
                        nc.vector.tensor_tensor(y[:], y[:], mask, A.mult)
                    return y

                # ========== conv1 (one-tile-lookahead pipeline) ==========
                y1v = None
                tiles1 = _hwtiles(HW1)
                heads = {}
                for ti in range(len(tiles1) + 1):
                    if ti < len(tiles1):
                        p0, pw = tiles1[ti]
                        pm = patchp.tile([128, CKK], CT, name="pm1",
                                         tag="pm1")[:pw]
                        _dma_patch(nc, pm, mp, p0 // 32, pw // 32)
                        heads[ti] = (p0, pw, bounds_head(pw, pm))
                    if ti == 1:
                        load_value_weights(1)
                        y1v = value_conv(1, xpc, HW1, 1, None)
                        vm = smallp.tile([32, HW1], DT.float32, name="vm",
                                         tag="vm")
                        nc.sync.dma_start(vm[:], vmask.ap())
                        nc.vector.tensor_tensor(y1v[:], y1v[:], vm[:], A.mult)
                        nc.sync.dma_start(
                            AP(tensor=cvv, offset=1,
                               ap=[[ROWS1 * 34, 32], [34, ROWS1], [1, 32]]),
                            y1v[:])
                    if ti - 1 not in heads:
                        continue
                    p0, pw, btl = heads.pop(ti - 1)
                    y0 = p0 // 32
                    nrows = pw // 32
                    lm = smallp.tile([128, 1], DT.float32, name="lm",
                                     tag="lm")[:pw]
                    nc.sync.dma_start(lm[:], lmask.ap()[p0:p0 + pw, :])
                    dl1, du1 = bounds_tail(1, pw, btl, None, lm)
                    m2 = smallp.tile([128, 32], CT, name="m2", tag="m2")[:pw]
                    h2 = smallp.tile([128, 32], CT, name="h2", tag="h2")[:pw]
                    nc.vector.tensor_tensor(m2[:], dl1[:], du1[:], A.add)
                    nc.vector.tensor_scalar(m2[:], m2[:], 0.5, None, A.mult)
                    nc.vector.tensor_tensor(h2[:], du1[:], dl1[:], A.subtract)
                    nc.vector.tensor_scalar(h2[:], h2[:], 0.5, None, A.mult)
                    nc.sync.dma_start(_canvas_interior(cm, y0, nrows), m2[:])
                    nc.sync.dma_start(_canvas_interior(ch, y0, nrows), h2[:])
                nc.sync.dma_start(twb[:], wb2.ap())

                # ================= conv2 =================
                load_value_weights(2)
                y2v = value_conv(2, cvv, HW2, 2, None)
                xcct = smallp.tile([32, HW2], DT.float32, name="xcct", tag="xcct")
                nc.sync.dma_start(xcct[:], xcc.ap())
                nc.vector.tensor_tensor(y2v[:], y2v[:], xcct[:], A.add)
                nc.scalar.activation(y2v[:], y2v[:], F.Relu)
                nc.sync.dma_start(out_v.ap(), y2v[:])
                tiles2 = _hwtiles(HW2)
                heads = {}
                for ti in range(len(tiles2) + 1):
                    if ti < len(tiles2):
                        p0, pw = tiles2[ti]
                        pmid = patchp.tile([128, CKK], CT, name="pmid",
                                           tag="pmid")[:pw]
                        phh = patchp.tile([128, CKK], CT, name="phh",
                                          tag="phh")[:pw]
                        _dma_patch(nc, pmid, cm, p0 // 32, pw // 32)
                        _dma_patch(nc, phh, ch, p0 // 32, pw // 32)
                        heads[ti] = (p0, pw, bounds_head(pw, pmid), phh)
                    if ti - 1 not in heads:
                        continue
                    p0, pw, btl, phh = heads.pop(ti - 1)
                    dl2, du2 = bounds_tail(2, pw, btl, phh, None)
                    lct = smallp.tile([128, 32], DT.float32, name="lct",
                                      tag="lct")[:pw]
                    uct = smallp.tile([128, 32], DT.float32, name="uct",
                                      tag="uct")[:pw]
                    nc.sync.dma_start(lct[:], lch.ap()[p0:p0 + pw, :])
                    nc.sync.dma_start(uct[:], uch.ap()[p0:p0 + pw, :])
                    for k, (rt, resid) in enumerate(((dl2, lct), (du2, uct))):
                        ro = smallp.tile([128, 32], DT.float32, name="ro",
                                         tag="ro")[:pw]
                        nc.vector.tensor_tensor(ro[:], rt[:], resid[:], A.add)
                        nc.scalar.activation(ro[:], ro[:], F.Relu)
                        nc.sync.dma_start(out_b.ap()[k, p0:p0 + pw, :], ro[:])
    return nc


def _build_u3(h0, repeat=1):
    """u2 + conv1 even/odd split: (b±h0)^8 = E ∓ odd with E (all even powers
    of a = pm−w) computed on the TensorEngine via combined-binomial
    stationaries sharing the value path's px^j moving tensors, and
    odd = 8·h0·b·(g+c1)(g+c2)(g+c3), g = a², via three fused STT passes on
    DVE (the cubic's roots are real:  −h0², −(3±2√2)h0²).  conv1's relu is
    dropped exactly-ish (error ≤ 288·h0⁸ ≈ 3e−6 in the 8th-power sum).
    conv2 is unchanged from u2."""
    nc = bass.Bass("TRN2", target_bir_lowering=False, debug=False,
                   num_devices=8)
    mp = nc.dram_tensor("mp", [20, 34, 32], CT, kind="ExternalInput")
    xpc = nc.dram_tensor("xpc", [32, 20, 34], DT.float32, kind="ExternalInput")
    lch = nc.dram_tensor("lch", [HW2, 32], DT.float32, kind="ExternalInput")
    uch = nc.dram_tensor("uch", [HW2, 32], DT.float32, kind="ExternalInput")
    xcc = nc.dram_tensor("xcc", [32, HW2], DT.float32, kind="ExternalInput")
    lmask = nc.dram_tensor("lmask", [HW1, 1], DT.float32, kind="ExternalInput")
    vmask = nc.dram_tensor("vmask", [32, HW1], DT.float32, kind="ExternalInput")
    wb1 = nc.dram_tensor("wb1", [128, 32 * CKK], CT, kind="ExternalInput")
    wb2 = nc.dram_tensor("wb2", [128, 32 * CKK], CT, kind="ExternalInput")
    wj1 = nc.dram_tensor("wj1", [8, CKK, 32], DT.float32, kind="ExternalInput")
    wj2 = nc.dram_tensor("wj2", [8, CKK, 32], DT.float32, kind="ExternalInput")
    wje = nc.dram_tensor("wje", [8, CKK, 32], DT.float32, kind="ExternalInput")
    we0 = nc.dram_tensor("we0", [96, 32], DT.float32, kind="ExternalInput")
    eye32 = nc.dram_tensor("eye32", [32, 32], DT.float32, kind="ExternalInput")
    b01 = nc.dram_tensor("b01", [32, 1], DT.float32, kind="ExternalInput")
    b02 = nc.dram_tensor("b02", [32, 1], DT.float32, kind="ExternalInput")
    cm = nc.dram_tensor("cm", [ROWS1, 34, 32], CT)
    ch = nc.dram_tensor("ch", [ROWS1, 34, 32], CT)
    cvv = nc.dram_tensor("cvv", [32, ROWS1, 34], DT.float32)
    out_b = nc.dram_tensor("out_b", [2, HW2, 32], DT.float32,
                           kind="ExternalOutput")
    out_v = nc.dram_tensor("out_v", [32, HW2], DT.float32,
                           kind="ExternalOutput")

    FATW = 32 * CKK
    import math
    c1 = h0 * h0
    c2 = (3.0 + 2.0 * math.sqrt(2.0)) * h0 * h0
    c3 = (3.0 - 2.0 * math.sqrt(2.0)) * h0 * h0

    with tile.TileContext(nc) as tc:
        with (
            tc.tile_pool(name="const", bufs=1) as constp,
            tc.tile_pool(name="wpool", bufs=1) as wpool,
            tc.tile_pool(name="patch", bufs=2) as patchp,
            tc.tile_pool(name="fat", bufs=2) as fatp,
            tc.tile_pool(name="small", bufs=3) as smallp,
            tc.tile_pool(name="vpow", bufs=1) as vpowp,
            tc.tile_pool(name="psum", bufs=2, space="PSUM") as psump,
        ):
            twb = wpool.tile([128, FATW], CT, name="twb")
            nc.sync.dma_start(twb[:], wb1.ap())
            twjs = {}
            tb0 = {}
            twje = {}

            def load_value_weights(cv):
                wjt = {1: wj1, 2: wj2}[cv]
                bt = {1: b01, 2: b02}[cv]
                for j in range(8):
                    for ck in range(3):
                        t = wpool.tile([96, 32], DT.float32,
                                       name=f"wj{cv}_{j}_{ck}")
                        nc.sync.dma_start(
                            t[:], wjt.ap()[j, ck * 96:(ck + 1) * 96, :])
                        twjs[(cv, j, ck)] = t
                        if cv == 1:
                            te = wpool.tile([96, 32], DT.float32,
                                            name=f"wje_{j}_{ck}")
                            nc.sync.dma_start(
                                te[:], wje.ap()[j, ck * 96:(ck + 1) * 96, :])
                            twje[(j, ck)] = te
                t = wpool.tile([32, 1], DT.float32, name=f"b0_{cv}")
                nc.sync.dma_start(t[:], bt.ap())
                tb0[cv] = t
                if cv == 1:
                    t = wpool.tile([96, 32], DT.float32, name="we0")
                    nc.sync.dma_start(t[:], we0.ap())
                    twje["j0"] = t
            zcol = constp.tile([128, 1], DT.float32, name="zcol")
            nc.gpsimd.memset(zcol[:], 0.0)
            zfill = constp.tile([128, 153], CT, name="zfill")
            nc.gpsimd.memset(zfill[:], 0.0)
            zfill32 = constp.tile([128, 153], DT.float32, name="zfill32")
            nc.gpsimd.memset(zfill32[:], 0.0)
            ones96 = constp.tile([96, HW1], DT.float32, name="ones96")
            nc.gpsimd.memset(ones96[:], 1.0)
            eye = constp.tile([32, 32], DT.float32, name="eye")
            nc.sync.dma_start(eye[:], eye32.ap())
            for _repidx in range(repeat):
                for t in (cm, ch):
                    nc.sync.dma_start(
                        AP(tensor=t, offset=0, ap=[[1, ROWS1 * 34 * 32]]),
                        zfill[:])
                nc.sync.dma_start(
                    AP(tensor=cvv, offset=0, ap=[[1, 32 * ROWS1 * 34]]),
                    zfill32[:])

                sttscr = constp.tile([128, CKK], CT, name="sttscr")

                def _sl(t, o):
                    return t[:, o * CKK:(o + 1) * CKK]

                def _as3(t):
                    return AP(tensor=t.tensor, offset=t.offset,
                              ap=[list(t.ap[0]), [CKK, 32], [1, CKK]])

                def bounds_head(pw, pm):
                    a = fatp.tile([128, FATW], CT, name="fatA",
                                  tag="fatA")[:pw]
                    b = fatp.tile([128, FATW], CT, name="fatB",
                                  tag="fatB")[:pw]
                    nc.vector.tensor_tensor(_as3(a), _rep(pm, 32),
                                            _as3(twb[:pw]), A.subtract)
                    nc.scalar.activation(b[:], a[:], F.Abs)
                    return a, b

                def bounds_head1(pw, pm):
                    """conv1 u3 head: a, b = |a|, g = a²."""
                    a, b = bounds_head(pw, pm)
                    g = fatp.tile([128, FATW], CT, name="fatC", tag="fatC")[:pw]
                    nc.scalar.activation(g[:], a[:], F.Square)
                    return b, g

                epsums = []
                esb = None

                def bounds_tail1(p0, pw, b, g, lm):
                    """conv1 u3 tail: odd chain + E combine + roots."""
                    w1t = fatp.tile([128, FATW], CT, name="fatD", tag="fatD",
                                    bufs=1)[:pw]
                    w2t = fatp.tile([128, FATW], CT, name="fatE", tag="fatE",
                                    bufs=1)[:pw]
                    acc = smallp.tile([128, 32], DT.float32, name="acc",
                                      tag="acc")[:pw]
                    scr = sttscr[:pw]
                    # w1 = (g+c1)*b ; w2 = (g+c2)*w1 ; acc_o = Σ_k (g+c3)*w2
                    nc.vector.scalar_tensor_tensor(
                        w1t[:], g[:], c1, b[:], A.add, A.mult)
                    nc.vector.scalar_tensor_tensor(
                        w2t[:], g[:], c2, w1t[:], A.add, A.mult)
                    for o in range(32):
                        nc.vector.scalar_tensor_tensor(
                            scr[:], _sl(g, o), c3, _sl(w2t, o), A.add, A.mult,
                            accum_out=acc[:, o:o + 1])
                    # E segment -> PSUM transposed [pw, 32]
                    et = psump.tile([128, 32], DT.float32, name="ET",
                                    tag="ET")[:pw]
                    nc.tensor.transpose(et[:], esb[:, p0:p0 + pw], eye[:])
                    roots = []
                    for sgn in (-1.0, 1.0):
                        z8 = smallp.tile([128, 32], DT.float32, name="z8",
                                         tag="z8")[:pw]
                        nc.vector.scalar_tensor_tensor(
                            z8[:], acc[:], sgn * 8.0 * h0, et[:],
                            A.mult, A.add)
                        lnz = smallp.tile([128, 32], DT.float32, name="lnz",
                                          tag="lnz")[:pw]
                        rt = smallp.tile([128, 32], CT, name="rt1",
                                         tag="rt1")[:pw]
                        nc.scalar.activation(lnz[:], z8[:], F.Ln)
                        nc.scalar.activation(rt[:], lnz[:], F.Exp,
                                             bias=lm[:], scale=0.125)
                        roots.append(rt)
                    return roots

                def bounds_tail(conv, pw, btl, ph, lm):
                    (b,) = btl
                    c = fatp.tile([128, FATW], CT, name="fatC", tag="fatC")
                    d = fatp.tile([128, FATW], CT, name="fatD", tag="fatD",
                                  bufs=1)
                    e = fatp.tile([128, FATW], CT, name="fatE", tag="fatE",
                                  bufs=1)
                    zl = smallp.tile([128, 32], DT.float32, name="zl", tag="zl")
                    zu = smallp.tile([128, 32], DT.float32, name="zu", tag="zu")
                    c, d, e = c[:pw], d[:pw], e[:pw]
                    zl, zu = zl[:pw], zu[:pw]
                    scr = sttscr[:pw]
                    sl = _sl
                    as3 = _as3
                    nc.vector.tensor_tensor(as3(c), as3(b), _rep(ph, 32),
                                            A.subtract)             # q
                    nc.vector.tensor_tensor(as3(d), as3(b), _rep(ph, 32),
                                            A.add)                  # s
                    nc.vector.tensor_scalar(c[:], c[:], 0.0, None, A.max)
                    nc.scalar.activation(e[:], c[:], F.Square)      # r2
                    nc.scalar.activation(c[:], e[:], F.Square)      # r4
                    nc.scalar.activation(e[:], d[:], F.Square)      # s2
                    nc.scalar.activation(d[:], e[:], F.Square)      # s4
                    rsum, ssum = c, d
                    for o in range(32):
                        nc.vector.scalar_tensor_tensor(
                            scr[:], sl(rsum, o), 0.0, sl(rsum, o), A.add, A.mult,
                            accum_out=zl[:, o:o + 1])
                    for o in range(32):
                        nc.vector.scalar_tensor_tensor(
                            scr[:], sl(ssum, o), 0.0, sl(ssum, o), A.add, A.mult,
                            accum_out=zu[:, o:o + 1])
                    roots = []
                    for z in (zl, zu):
                        lnz = smallp.tile([128, 32], DT.float32, name="lnz",
                                          tag="lnz")[:pw]
                        rt = smallp.tile([128, 32], DT.float32, name=f"rt{conv}",
                                         tag=f"rt{conv}")[:pw]
                        nc.scalar.activation(lnz[:], z[:], F.Ln)
                        nc.scalar.activation(rt[:], lnz[:], F.Exp,
                                             bias=zcol[:pw], scale=0.125)
                        roots.append(rt)
                    return roots

                def value_conv(conv, src_dram, hw, wtile_key, mask,
                               emit_e=False):
                    nrows = hw // 32
                    px = []
                    for dy in range(3):
                        t = vpowp.tile([96, hw], DT.float32, name=f"px{dy}",
                                       tag=f"px{dy}")
                        for dx in range(3):
                            src = AP(tensor=src_dram,
                                     offset=dy * 34 + dx,
                                     ap=[[20 * 34 if conv == 1 else ROWS1 * 34, 32],
                                         [34, nrows], [1, 32]])
                            nc.sync.dma_start(t[dx * 32:(dx + 1) * 32, :], src)
                        px.append(t)
                    nps = (hw + 511) // 512
                    psums = [psump.tile([32, min(512, hw - i * 512)], DT.float32,
                                        name=f"vps{i}", tag=f"vps{i}")
                             for i in range(nps)]
                    if emit_e:
                        epsums.clear()
                        epsums.extend(
                            psump.tile([32, min(512, hw - i * 512)], DT.float32,
                                       name=f"eps{i}", tag=f"eps{i}", bufs=1)
                            for i in range(nps))

                    def mm(j, ck, t, start):
                        for i, ps in enumerate(psums):
                            nc.tensor.matmul(
                                ps[:], twjs[(conv, j, ck)][:],
                                t[:, i * 512:i * 512 + ps.shape[1]],
                                start=start, stop=(j == 7))
                        if emit_e:
                            for i, ps in enumerate(epsums):
                                nc.tensor.matmul(
                                    ps[:], twje[(j, ck)][:],
                                    t[:, i * 512:i * 512 + ps.shape[1]],
                                    start=(start and j == 0), stop=False)
                    for ck in range(3):
                        p1 = px[ck]
                        p2 = vpowp.tile([96, hw], DT.float32, name="p2", tag="p2")
                        p4 = vpowp.tile([96, hw], DT.float32, name="p4", tag="p4")
                        tmp = vpowp.tile([96, hw], DT.float32, name="tmp",
                                         tag="tmp")
                        mm(0, ck, p1, start=(ck == 0))
                        nc.vector.tensor_tensor(p2[:], p1[:], p1[:], A.mult)
                        mm(1, ck, p2, start=False)
                        nc.vector.tensor_tensor(tmp[:], p2[:], p1[:], A.mult)
                        mm(2, ck, tmp, start=False)
                        nc.vector.tensor_tensor(p4[:], p2[:], p2[:], A.mult)
                        mm(3, ck, p4, start=False)
                        nc.vector.tensor_tensor(tmp[:], p4[:], p1[:], A.mult)
                        mm(4, ck, tmp, start=False)
                        nc.vector.tensor_tensor(tmp[:], p4[:], p2[:], A.mult)
                        mm(5, ck, tmp, start=False)
                        nc.vector.tensor_tensor(tmp[:], tmp[:], p1[:], A.mult)
                        mm(6, ck, tmp, start=False)
                        nc.vector.tensor_tensor(tmp[:], p4[:], p4[:], A.mult)
                        mm(7, ck, tmp, start=False)
                    if emit_e:
                        for i, ps in enumerate(epsums):
                            nc.tensor.matmul(
                                ps[:], twje["j0"][:],
                                ones96[:, i * 512:i * 512 + ps.shape[1]],
                                start=False, stop=True)
                    y = smallp.tile([32, HW1], DT.float32, name=f"yv{conv}",
                                    tag=f"yv{conv}", bufs=1)[:, :hw]
                    for i, ps in enumerate(psums):
                        w = ps.shape[1]
                        seg = y[:, i * 512:i * 512 + w]
                        nc.scalar.activation(seg, ps[:], F.Relu)
                        nc.scalar.activation(seg, seg, F.Ln, bias=tb0[conv][:])
                        nc.scalar.activation(seg, seg, F.Exp, scale=0.125)
                    if mask is not None:
                        nc.vector.tensor_tensor(y[:], y[:], mask, A.mult)
                    return y

                # ========== conv1 (one-tile-lookahead pipeline) ==========
                y1v = None
                tiles1 = _hwtiles(HW1)
                heads = {}
                for ti in range(len(tiles1) + 1):
                    if ti < len(tiles1):
                        p0, pw = tiles1[ti]
                        pm = patchp.tile([128, CKK], CT, name="pm1",
                                         tag="pm1")[:pw]
                        _dma_patch(nc, pm, mp, p0 // 32, pw // 32)
                        heads[ti] = (p0, pw, bounds_head1(pw, pm))
                    if ti == 1:
                        load_value_weights(1)
                        y1v = value_conv(1, xpc, HW1, 1, None, emit_e=True)
                        esb = smallp.tile([32, HW1], DT.float32, name="esb",
                                          tag="esb", bufs=1)
                        for i, ps in enumerate(epsums):
                            w = ps.shape[1]
                            nc.scalar.copy(esb[:, i * 512:i * 512 + w], ps[:])
                        vm = smallp.tile([32, HW1], DT.float32, name="vm",
                                         tag="vm", bufs=1)
                        nc.sync.dma_start(vm[:], vmask.ap())
                        nc.vector.tensor_tensor(y1v[:], y1v[:], vm[:], A.mult)
                        nc.sync.dma_start(
                            AP(tensor=cvv, offset=1,
                               ap=[[ROWS1 * 34, 32], [34, ROWS1], [1, 32]]),
                            y1v[:])
                    if ti - 1 not in heads:
                        continue
                    p0, pw, (bt, gt) = heads.pop(ti - 1)
                    y0 = p0 // 32
                    nrows = pw // 32
                    lm = smallp.tile([128, 1], DT.float32, name="lm",
                                     tag="lm")[:pw]
                    nc.sync.dma_start(lm[:], lmask.ap()[p0:p0 + pw, :])
                    dl1, du1 = bounds_tail1(p0, pw, bt, gt, lm)
                    m2 = smallp.tile([128, 32], CT, name="m2", tag="m2")[:pw]
                    h2 = smallp.tile([128, 32], CT, name="h2", tag="h2")[:pw]
                    nc.vector.tensor_tensor(m2[:], dl1[:], du1[:], A.add)
                    nc.vector.tensor_scalar(m2[:], m2[:], 0.5, None, A.mult)
                    nc.vector.tensor_tensor(h2[:], du1[:], dl1[:], A.subtract)
                    nc.vector.tensor_scalar(h2[:], h2[:], 0.5, None, A.mult)
                    nc.sync.dma_start(_canvas_interior(cm, y0, nrows), m2[:])
                    nc.sync.dma_start(_canvas_interior(ch, y0, nrows), h2[:])
                nc.sync.dma_start(twb[:], wb2.ap())

                # ================= conv2 =================
                load_value_weights(2)
                y2v = value_conv(2, cvv, HW2, 2, None)
                xcct = smallp.tile([32, HW2], DT.float32, name="xcct", tag="xcct",
                                   bufs=1)
                nc.sync.dma_start(xcct[:], xcc.ap())
                nc.vector.tensor_tensor(y2v[:], y2v[:], xcct[:], A.add)
                nc.scalar.activation(y2v[:], y2v[:], F.Relu)
                nc.sync.dma_start(out_v.ap(), y2v[:])
                tiles2 = _hwtiles(HW2)
                heads = {}
                for ti in range(len(tiles2) + 1):
                    if ti < len(tiles2):
                        p0, pw = tiles2[ti]
                        pmid = patchp.tile([128, CKK], CT, name="pmid",
                                           tag="pmid")[:pw]
                        phh = patchp.tile([128, CKK], CT, name="phh",
                                          tag="phh")[:pw]
                        _dma_patch(nc, pmid, cm, p0 // 32, pw // 32)
                        _dma_patch(nc, phh, ch, p0 // 32, pw // 32)
                        heads[ti] = (p0, pw, bounds_head(pw, pmid)[1:], phh)
                    if ti - 1 not in heads:
                        continue
                    p0, pw, btl, phh = heads.pop(ti - 1)
                    dl2, du2 = bounds_tail(2, pw, btl, phh, None)
                    lct = smallp.tile([128, 32], DT.float32, name="lct",
                                      tag="lct")[:pw]
                    uct = smallp.tile([128, 32], DT.float32, name="uct",
                                      tag="uct")[:pw]
                    nc.sync.dma_start(lct[:], lch.ap()[p0:p0 + pw, :])
                    nc.sync.dma_start(uct[:], uch.ap()[p0:p0 + pw, :])
                    for k, (rt, resid) in enumerate(((dl2, lct), (du2, uct))):
                        ro = smallp.tile([128, 32], DT.float32, name="ro",
                                         tag="ro")[:pw]
                        nc.vector.tensor_tensor(ro[:], rt[:], resid[:], A.add)
                        nc.scalar.activation(ro[:], ro[:], F.Relu)
                        nc.sync.dma_start(out_b.ap()[k, p0:p0 + pw, :], ro[:])
    return nc


_CACHE = {}


def _get_nc(repeat=1):
    key = f"nc{repeat}"
    if key not in _CACHE:
        _CACHE[key] = _build(repeat)
    return _CACHE[key]


def _norm_w(w):
    """[32,32,3,3] -> [32,288] mean-normalized, (dy,dx,c)-ordered."""
    wf = w.reshape(32, -1).astype(np.float32)
    wf = wf - wf.mean(axis=1, keepdims=True)
    return np.ascontiguousarray(
        wf.reshape(32, 32, 3, 3).transpose(0, 2, 3, 1).reshape(32, 288))


def _w_expand(wn):
    """[32,288] -> [128, 32*288] partition-broadcast, CT."""
    row = wn.reshape(1, 32 * 288)
    return np.ascontiguousarray(
        np.broadcast_to(row, (128, 32 * 288))).astype(NPCT)


def _prep_in_maps(x, weight1, weight2, lower=None, upper=None):
    x = np.asarray(x, np.float32)
    global _PREP_LU
    _PREP_LU = (np.asarray(lower, np.float32) if lower is not None else x - EPS,
                np.asarray(upper, np.float32) if upper is not None else x + EPS)
    wn1 = _norm_w(np.asarray(weight1, np.float32))
    wn2 = _norm_w(np.asarray(weight2, np.float32))
    w1 = _w_expand(wn1)
    w2 = _w_expand(wn2)
    from math import comb
    def wj_of(wn):
        # wj[j-1][k, o] = C(8,j) * (-w[o,k])^(8-j), j = 1..8
        out = np.zeros((8, CKK, 32), np.float32)
        for j in range(1, 9):
            out[j - 1] = (comb(8, j) * (-wn.T) ** (8 - j)).astype(np.float32)
        return out
    wj1 = wj_of(wn1)
    wj2 = wj_of(wn2)
    b01 = (wn1.astype(np.float64) ** 8).sum(1).astype(np.float32).reshape(32, 1)
    b02 = (wn2.astype(np.float64) ** 8).sum(1).astype(np.float32).reshape(32, 1)

    in_maps = []
    lo, up = _PREP_LU
    m1 = (lo + up) * 0.5
    h1 = (up - lo) * 0.5
    for core in range(8):
        b, half = core // 2, core % 2
        r0 = half * 16
        mp = np.zeros((20, 34, 32), np.float32)
        hpav = np.zeros((20, 34, 32), np.float32)
        xpcc = np.zeros((32, 20, 34), np.float32)
        for i in range(20):
            a = r0 - 2 + i
            if 0 <= a < H:
                mp[i, 1:33, :] = m1[b, :, a, :].T
                hpav[i, 1:33, :] = h1[b, :, a, :].T
                xpcc[:, i, 1:33] = x[b, :, a, :]
        lch = np.ascontiguousarray(
            lo[b, :, r0:r0 + 16, :].transpose(1, 2, 0).reshape(HW2, 32))
        uch = np.ascontiguousarray(
            up[b, :, r0:r0 + 16, :].transpose(1, 2, 0).reshape(HW2, 32))
        xcc = np.ascontiguousarray(
            x[b, :, r0:r0 + 16, :].reshape(32, HW2))
        lm = np.zeros((HW1, 1), np.float32)
        vm = np.ones((32, HW1), np.float32)
        if half == 0:
            lm[:32] = NEGINF
            vm[:, :32] = 0.0
        else:
            lm[-32:] = NEGINF
            vm[:, -32:] = 0.0
        in_maps.append({
            "mp": mp.astype(NPCT), "hp": hpav.astype(NPCT), "xpc": xpcc,
            "lch": lch, "uch": uch, "xcc": xcc,
            "lmask": lm, "vmask": vm,
            "wb1": w1, "wb2": w2, "wj1": wj1, "wj2": wj2,
            "b01": b01, "b02": b02,
        })
    return in_maps


def _unshard(results):
    full = np.zeros((3, B, C, H, W), np.float32)
    for core in range(8):
        b, half = core // 2, core % 2
        r0 = half * 16
        ob = results[core]["out_b"]           # [2, 512, 32] (hw, c)
        ov = results[core]["out_v"]           # [32, 512]    (c, hw)
        full[0, b, :, r0:r0 + 16, :] = ov.reshape(32, 16, 32)
        full[1:, b, :, r0:r0 + 16, :] = (
            ob.reshape(2, 16, 32, 32).transpose(0, 3, 1, 2))
    return full


def _get_nc_u2(h0, repeat=1):
    key = f"ncu3_{repeat}_{h0:.9e}"
    if key not in _CACHE:
        _CACHE[key] = _build_u3(h0, repeat)
    return _CACHE[key]


def _prep_wje(wn, h0):
    """Combined-binomial stationaries for E = Σ_m C(8,2m) h0^(8-2m) Σ_k a^2m.

    wje[jj][k,o] pairs with moving px^(jj+1); we0[k,o] = bE[o]/96 pairs with
    a ones moving tensor (bE = j=0 terms incl. 288·h0⁸)."""
    from math import comb
    w64 = wn.astype(np.float64)
    coef = [comb(8, 2 * m) * h0 ** (8 - 2 * m) for m in range(5)]
    wje = np.zeros((8, CKK, 32), np.float64)
    for jj in range(8):
        j = jj + 1
        for m in range(5):
            if 2 * m >= j:
                wje[jj] += coef[m] * comb(2 * m, j) * (-w64.T) ** (2 * m - j)
    bE = sum(coef[m] * (w64 ** (2 * m)).sum(1) for m in range(5))    # [32]
    we0 = np.broadcast_to(bE[None, :] / 96.0, (96, 32))
    return (wje.astype(np.float32),
            np.ascontiguousarray(we0).astype(np.float32))


def _prep_in_maps_u2(x, weight1, weight2, lower, upper):
    maps = _prep_in_maps(x, weight1, weight2, lower, upper)
    lo = np.asarray(lower, np.float32)
    up = np.asarray(upper, np.float32)
    h0 = float(np.median((up - lo) * 0.5))
    wn1 = _norm_w(np.asarray(weight1, np.float32))
    wje, we0 = _prep_wje(wn1, h0)
    eye = np.eye(32, dtype=np.float32)
    for m in maps:
        del m["hp"]
        m["wje"] = wje
        m["we0"] = we0
        m["eye32"] = eye
    return maps


def kernel(x, lower, upper, weight1, weight2):
    lo = np.asarray(lower, np.float32)
    up = np.asarray(upper, np.float32)
    hv = (up - lo) * 0.5
    h0 = float(np.median(hv))
    uniform = (h0 > 0 and
               float(np.max(np.abs(hv - h0))) <= max(1e-5, 1e-3 * h0))
    if uniform:
        in_maps = _prep_in_maps_u2(x, weight1, weight2, lo, up)
        nc = _get_nc_u2(h0)
    else:
        in_maps = _prep_in_maps(x, weight1, weight2, lo, up)
        nc = _get_nc()
    res = run_bass_kernel_spmd(nc, in_maps, list(range(8)))
    _CACHE["last_results"] = res
    return _unshard(res.results)

